# revision 1
# baseline (speedup 1.0000x reference)
"""Block-global self-attention Trainium2 kernel (SPMD over 8 NeuronCores).

Sharding: core c -> batch n = c//4, heads h0 = (c%4)*4 .. h0+3.
Each core receives x = hidden[n] [4096,2048] and wq/wk/wv = W[:, cols]
[2048,512], returns out [4096,512] (its head-column stripe of batch n).

Per-core pipeline:
  P: bf16 projections -> qT/kT [d,t] + V2 (t-major, 64-row-shifted so local
     windows are two aligned full-K tiles); fp32 approx q-norms -> grid.
  A: local block attention; softmax without max-subtraction (|score|<8);
     probs kept unnormalized bf16, 1/denom fused into the final ACT copy.
  B: exact top-62 global tokens via: packed-value (quantized norm + token id
     in low mantissa) 3-level max8 top-96 candidates -> indirect-gather
     X rows -> exact fp32 norms -> 62nd threshold (bos/eos forced slots)
     -> sorted final index list -> global attention -> indirect row scatter
     (replace; any duplicate rows carry identical values).
"""
import os
import numpy as np

import concourse.bass as bass
import concourse.bacc as bacc
import concourse.mybir as mybir
from concourse.tile import TileContext, add_dep_helper
from concourse.bass_utils import run_bass_kernel_spmd

F32 = mybir.dt.float32
BF16 = mybir.dt.bfloat16
I32 = mybir.dt.int32

T = 4096
H = 2048
D = 128
NH = 4
KO = H // 128
NB = T // 128
CW = 512
NCHUNK = T // CW
NEG = -30.0
NEGRAW = -30.0 * float(np.sqrt(128.0))  # pre-divided by ACT scale
SCALE = float(1.0 / np.sqrt(128.0))
NCAND = 96
NSLOT = NCAND + 2
NIDX = 66
DEBUG = bool(int(os.environ.get("KERNEL_DEBUG", "0")))


def ts(i, sz):
    return slice(i * sz, (i + 1) * sz)


def _raw(inst):
    return inst.ins if hasattr(inst, "ins") else inst


def build_program():
    nc = bacc.Bacc("TRN2", target_bir_lowering=False, debug=False,
                   enable_asserts=True)
    x_d = nc.dram_tensor("x", (T, H), F32, kind="ExternalInput").ap()
    xt_d = nc.dram_tensor("xt", (H, T), F32, kind="ExternalInput").ap()
    wq_d = nc.dram_tensor("wq", (H, NH * D), F32, kind="ExternalInput").ap()
    wk_d = nc.dram_tensor("wk", (H, NH * D), F32, kind="ExternalInput").ap()
    wv_d = nc.dram_tensor("wv", (H, NH * D), F32, kind="ExternalInput").ap()
    id_d = nc.dram_tensor("ident", (128, 128), F32, kind="ExternalInput").ap()
    out_d = [nc.dram_tensor(f"out{h}", (T, D), F32, kind="ExternalOutput").ap()
             for h in range(NH)]
    dbg = {}
    if DEBUG:
        dbg["na"] = nc.dram_tensor("dbg_na", (128, NH, 32), F32, kind="ExternalOutput").ap()
        dbg["cand"] = nc.dram_tensor("dbg_cand", (NH, NSLOT), F32, kind="ExternalOutput").ap()
        dbg["ne"] = nc.dram_tensor("dbg_ne", (NH, NSLOT), F32, kind="ExternalOutput").ap()
        dbg["sidx"] = nc.dram_tensor("dbg_sidx", (NSLOT, NH), I32, kind="ExternalOutput").ap()

    with TileContext(nc) as tc:
        const = tc.alloc_tile_pool(name="const", bufs=1)
        res = tc.alloc_tile_pool(name="res", bufs=1)
        dram = tc.alloc_tile_pool(name="dram", bufs=1, space="DRAM")

        ident = const.tile([128, 128], F32)
        nc.sync.dma_start(ident[:], id_d)
        identb = const.tile([128, 128], BF16)
        nc.vector.tensor_copy(identb[:], ident[:])
        ones_b = const.tile([128, 1], BF16)
        nc.vector.memset(ones_b[:], 1.0)
        ones = const.tile([128, 1], F32)
        nc.vector.memset(ones[:], 1.0)
        iota_g = const.tile([128, NH, 32], F32)
        nc.gpsimd.iota(iota_g[:], pattern=[[0, NH], [1, 32]], base=0,
                       channel_multiplier=32, allow_small_or_imprecise_dtypes=True)

        qT = [res.tile([128, T], BF16, tag=f"qT{h}", name=f"qT{h}") for h in range(NH)]
        kT = [res.tile([128, 64 + T + 64], BF16, tag=f"kT{h}", name=f"kT{h}") for h in range(NH)]
        V2 = res.tile([128, NB + 1, NH, D + 1], BF16, tag="V2")
        wqb = res.tile([128, KO, NH * D], BF16, tag="wqb")
        nagrid = res.tile([128, NH, 32], F32, tag="nagrid")
        na_dram = dram.tile([NH, T], F32)

        # ---------------- pools ----------------
        psum = tc.alloc_tile_pool(name="psum", bufs=1, space="PSUM")
        ab = tc.alloc_tile_pool(name="ab", bufs=4)

        def psA2k(nm):   # 2KB f32 one-shot psums
            t = psum.tile([128, 512], F32, tag="A2k", bufs=2, name=nm)
            return t
        def psTbf(nm):   # bf16 transpose targets
            t = psum.tile([128, 512], BF16, tag="Tbf", bufs=2, name=nm)
            return t
        def psBLK(nm):   # per-block S + ctx combined (and global Sg)
            t = psum.tile([128, 512], F32, tag="blk", bufs=2, name=nm)
            return t
        def psACC(nm):   # held accumulators
            t = psum.tile([128, 512], F32, tag="ACC", bufs=2, name=nm)
            return t

        # ---------------- interleaved: local attention + global per head ----------------
        out_write_insts = [[] for _ in range(NH)]

        def local_block(h, b):
            blk = psBLK("blk")
            # S^T halves: [tk(128), tq(128)]; half g covers window pos g*128..,
            # i.e. k tokens [b*128 - 64 + g*128, ...). kT is 64-padded.
            for g in range(2):
                seg = b + g
                nc.tensor.matmul(blk[:, g * 128:(g + 1) * 128],
                                 kT[h][:, seg * 128:seg * 128 + 128],
                                 qT[h][:, ts(b, 128)], start=True, stop=True)
            if b == 0:
                nc.vector.memset(blk[0:64, 0:128], NEGRAW)
            if b == NB - 1:
                nc.vector.memset(blk[64:128, 128:256], NEGRAW)
            PT = ab.tile([128, 256], BF16, tag="PT", name="PT", bufs=2)
            nc.scalar.activation(PT[:], blk[:, 0:256], mybir.ActivationFunctionType.Exp,
                                 scale=SCALE)
            pC = blk[:, 256:385]
            nc.tensor.matmul(pC, PT[:, 0:128], V2[:, b, h, :],
                             start=True, stop=False)
            nc.tensor.matmul(pC, PT[:, 128:256], V2[:, b + 1, h, :],
                             start=False, stop=True)
            rc = ab.tile([128, 1], F32, tag="rc", name="rc", bufs=8)
            nc.vector.reciprocal(rc[:], pC[:, 128:129])
            co = ab.tile([128, 128], F32, tag="co", name="co", bufs=3)
            nc.vector.tensor_scalar_mul(co[:], pC[:, 0:128], rc[:])
            w = nc.sync.dma_start(out_d[h][ts(b, 128), :], co[:])
            out_write_insts[h].append(_raw(w))

        def global_head(h):
            qgT = qgTh[h]
            Pg = gbig.tile([128, 64 + T + 64], BF16, tag="Pg", name="Pg", bufs=1)
            nc.vector.memset(Pg[96:128, :], 0.0)
            nc.vector.memset(Pg[0:96, 0:64], 0.0)
            nc.vector.memset(Pg[0:96, 64 + T:], 0.0)
            for j in range(8):
                psg = psBLK("psg")[:NSLOT, :]
                nc.tensor.matmul(psg, qgT[:], kT[h][:, 64 + j * 512:64 + (j + 1) * 512],
                                 start=True, stop=True)
                nc.scalar.activation(Pg[0:NSLOT, 64 + j * 512:64 + (j + 1) * 512], psg,
                                     mybir.ActivationFunctionType.Exp,
                                     scale=SCALE)

            pgc = psACC("pgc")[:NSLOT, :D + 1]
            for j in range(NB + 1):
                ppg = psTbf("ppg")[:, :128]
                nc.tensor.transpose(ppg, Pg[:, j * 128:j * 128 + 128], identb[:])
                pgt = gw.tile([128, 128], BF16, tag="pgt")
                nc.vector.tensor_copy(pgt[:], ppg)
                nc.tensor.matmul(pgc[:], pgt[:, 0:NSLOT], V2[:, j, h, :],
                                 start=(j == 0), stop=(j == NB),
                                 skip_group_check=True)
            rcg = gw.tile([NSLOT, 1], F32, tag="rcg")
            nc.vector.reciprocal(rcg[:], pgc[:, D:D + 1])
            gco = gw.tile([NSLOT, 128], F32, tag="gco")
            nc.vector.tensor_scalar_mul(gco[:], pgc[:, 0:D], rcg[:])
            scat = nc.gpsimd.indirect_dma_start(
                out=out_d[h][:],
                out_offset=bass.IndirectOffsetOnAxis(ap=sidx_i[:, h:h + 1], axis=0),
                in_=gco[:], in_offset=None,
                bounds_check=4095, oob_is_err=False)
            for w in out_write_insts[h]:
                add_dep_helper(_raw(scat), w, reason="scatter after local writes")


        A_DONE = [0]
        # ---------------- phase P ----------------
        na_writes = []
        wkv = tc.alloc_tile_pool(name="wkv", bufs=1)
        wkb = wkv.tile([128, KO, NH * D], BF16, tag="wkb")
        wvb = wkv.tile([128, KO, NH * D], BF16, tag="wvb")
        wb = {"q": wqb, "k": wkb, "v": wvb}

        with tc.tile_pool(name="pp", bufs=2) as pp, \
             tc.tile_pool(name="pp1", bufs=1) as pp1:

            # weights via Pool DMA queue (keeps SP free for x), 256-col slices
            for nm, wd in (("q", wq_d), ("k", wk_d), ("v", wv_d)):
                wr = wd.rearrange("(ko p) m -> p ko m", p=128)
                for kb in range(KO):
                    wstg = pp.tile([128, 1, NH * D], F32, tag="wstg")
                    nc.gpsimd.dma_start(wstg[:], wr[:, kb:kb + 1, :])
                    nc.vector.tensor_copy(wb[nm][:, kb:kb + 1, :], wstg[:])

            for h in range(NH):
                nc.vector.memset(kT[h][:, 0:64], 0.0)
                nc.vector.memset(kT[h][:, 64 + T:], 0.0)
            nc.vector.memset(V2[0:64, 0, :, :], 0.0)
            nc.vector.memset(V2[64:128, NB, :, :], 0.0)
            nc.vector.memset(V2[:, :, :, D:D + 1], 1.0)

            for c in range(NCHUNK):
                xtb = pp1.tile([128, KO, CW], BF16, tag="xtb", bufs=2)
                xtr = xt_d.rearrange("(ko p) t -> p ko t", p=128)
                for kg in range(4):
                    xts = pp.tile([128, 4, CW], F32, tag="xts", bufs=2)
                    nc.sync.dma_start(xts[:], xtr[:, kg * 4:(kg + 1) * 4, ts(c, CW)])
                    nc.vector.tensor_copy(xtb[:, kg * 4:(kg + 1) * 4, :], xts[:])
                for h in range(NH):
                    for nm, dstT in (("q", qT[h]), ("k", kT[h])):
                        ps = psA2k("psqk")
                        for kb in range(KO):
                            nc.tensor.matmul(ps[:], wb[nm][:, kb, ts(h, D)],
                                             xtb[:, kb, :], start=(kb == 0),
                                             stop=(kb == KO - 1))
                        off = 64 if nm == "k" else 0
                        nc.vector.tensor_copy(dstT[:, off + c * CW:off + (c + 1) * CW], ps[:])
                        if nm == "q":
                            sq = pp.tile([128, CW], BF16, tag="sq", bufs=1)
                            nc.vector.tensor_tensor(sq[:], dstT[:, ts(c, CW)],
                                                    dstT[:, ts(c, CW)],
                                                    op=mybir.AluOpType.mult)
                            pn = psA2k("pn")[:1, :]
                            nc.tensor.matmul(pn, ones_b[:], sq[:],
                                             start=True, stop=True)
                            narow = pp.tile([1, CW], F32, tag="narow", bufs=1)
                            nc.vector.tensor_copy(narow[:], pn)
                            w = nc.sync.dma_start(na_dram[h:h + 1, ts(c, CW)], narow[:])
                            na_writes.append(_raw(w))
                for s in range(CW // 128):
                    sg = c * (CW // 128) + s
                    pv = psA2k("psv")
                    for kb in range(KO):
                        nc.tensor.matmul(pv[:], xtb[:, kb, ts(s, 128)],
                                         wb["v"][:, kb, :], start=(kb == 0),
                                         stop=(kb == KO - 1))
                    vt = pp.tile([128, NH * D], BF16, tag="vtmp", bufs=1)
                    nc.vector.tensor_copy(vt[:], pv[:])
                    nc.sync.dma_start(V2[64:128, sg, :, 0:D],
                                      vt[0:64, :].rearrange("p (h d) -> p h d", h=NH))
                    nc.sync.dma_start(V2[0:64, sg + 1, :, 0:D],
                                      vt[64:128, :].rearrange("p (h d) -> p h d", h=NH))
                # interleave ready local-attention blocks (1-chunk lag)
                hi = min(4 * c - 2 + 1, NB)
                for b in range(A_DONE[0], hi):
                    for h in range(NH):
                        local_block(h, b)
                A_DONE[0] = max(A_DONE[0], hi)
        wkv.release()

        # ---------------- phase B part 1: candidates + exact topk ----------------
        gp = tc.alloc_tile_pool(name="gp", bufs=1)
        r = nc.sync.dma_start(nagrid[:],
                              na_dram[:].rearrange("h (p j) -> p h j", p=128))
        for w in na_writes:
            add_dep_helper(_raw(r), w, reason="na grid read after writes")

        m0 = gp.tile([128, NH, 32], F32)
        nc.vector.tensor_scalar(m0[:], iota_g[:], 0.0, scalar2=None,
                                op0=mybir.AluOpType.is_equal)
        m1 = gp.tile([128, NH, 32], F32)
        nc.vector.tensor_scalar(m1[:], iota_g[:], 4095.0, scalar2=None,
                                op0=mybir.AluOpType.is_equal)
        nc.vector.tensor_tensor(m0[:], m0[:], m1[:], op=mybir.AluOpType.add)
        nagp = gp.tile([128, NH, 32], F32)
        nc.vector.tensor_tensor(nagp[:], nagrid[:], m0[:], op=mybir.AluOpType.mult)
        nc.vector.tensor_tensor(nagp[:], nagrid[:], nagp[:], op=mybir.AluOpType.subtract)
        nc.vector.tensor_scalar_mul(m0[:], m0[:], 1.0e6)
        nc.vector.tensor_tensor(nagp[:], nagp[:], m0[:], op=mybir.AluOpType.subtract)
        pk = gp.tile([128, NH, 32], F32)
        nc.vector.tensor_scalar_mul(pk[:], nagp[:], 4.0)
        pki = gp.tile([128, NH, 32], I32)
        nc.vector.tensor_copy(pki[:], pk[:])
        nc.vector.tensor_copy(pk[:], pki[:])
        nc.vector.tensor_scalar_mul(pk[:], pk[:], 0.125)
        io16 = gp.tile([128, NH, 32], F32)
        nc.vector.tensor_scalar_mul(io16[:], iota_g[:], 2.0 ** -16)
        nc.vector.tensor_tensor(pk[:], pk[:], io16[:], op=mybir.AluOpType.add)
        pk2 = pk[:].rearrange("p h j -> p (h j)")

        cand1 = gp.tile([128, NH * 16], F32)
        for h in range(NH):
            for rr in range(2):
                mx = gp.tile([128, 8], F32, tag="mx1")
                nc.vector.max(out=mx[:], in_=pk2[:, ts(h, 32)])
                nc.vector.tensor_copy(cand1[:, h * 16 + rr * 8:h * 16 + rr * 8 + 8], mx[:])
                nc.vector.match_replace(out=pk2[:, ts(h, 32)], in_to_replace=mx[:],
                                        in_values=pk2[:, ts(h, 32)], imm_value=-1e30)
        lvl2 = gp.tile([64, 128], F32)
        for h in range(NH):
            for g in range(8):
                nc.sync.dma_start(lvl2[h * 16:(h + 1) * 16, ts(g, 16)],
                                  cand1[16 * g:16 * (g + 1), ts(h, 16)])
        cand2 = gp.tile([64, 24], F32)
        for rr in range(3):
            mx = gp.tile([64, 8], F32, tag="mx2")
            nc.vector.max(out=mx[:], in_=lvl2[:])
            nc.vector.tensor_copy(cand2[:, ts(rr, 8)], mx[:])
            nc.vector.match_replace(out=lvl2[:], in_to_replace=mx[:],
                                    in_values=lvl2[:], imm_value=-1e30)
        c2d = dram.tile([64, 24], F32)
        w2 = nc.sync.dma_start(c2d[:], cand2[:])
        lvl3 = gp.tile([NH, 384], F32)
        r3 = nc.sync.dma_start(lvl3[:],
                               c2d[:].rearrange("(h p) c -> h (p c)", h=NH))
        add_dep_helper(_raw(r3), _raw(w2), reason="lvl3 read after write")
        tops = gp.tile([NH, NCAND], F32)
        for rr in range(12):
            mx = gp.tile([NH, 8], F32, tag="mx3")
            nc.vector.max(out=mx[:], in_=lvl3[:])
            nc.vector.tensor_copy(tops[:, ts(rr, 8)], mx[:])
            nc.vector.match_replace(out=lvl3[:], in_to_replace=mx[:],
                                    in_values=lvl3[:], imm_value=-1e30)

        def decode_t(dst, src, n):
            t1 = gp.tile([NH, n], F32, tag="dec1")
            nc.vector.tensor_scalar_mul(t1[:], src, 8.0)
            t1i = gp.tile([NH, n], I32, tag="dec2")
            nc.vector.tensor_copy(t1i[:], t1[:])
            t1f = gp.tile([NH, n], F32, tag="dec3")
            nc.vector.tensor_copy(t1f[:], t1i[:])
            nc.vector.tensor_tensor(t1[:], t1[:], t1f[:], op=mybir.AluOpType.subtract)
            nc.vector.tensor_scalar_mul(dst, t1[:], 8192.0)

        cand_t = gp.tile([NH, NSLOT], F32)
        decode_t(cand_t[:, 0:NCAND], tops[:], NCAND)
        nc.vector.memset(cand_t[:, NCAND:NCAND + 1], 0.0)
        nc.vector.memset(cand_t[:, NCAND + 1:NSLOT], 4095.0)
        if DEBUG:
            nc.sync.dma_start(dbg["cand"], cand_t[:])

        # B pools (opened post-P; reuse P space)
        gbig = tc.alloc_tile_pool(name="gbig", bufs=2)
        gw = tc.alloc_tile_pool(name="gw", bufs=2)

        pslt = psA2k("pslt")[:NSLOT, :NH]
        nc.tensor.transpose(pslt, cand_t[:], ident[:NH, :NH])
        ctf = gp.tile([NSLOT, NH], F32)
        nc.vector.tensor_copy(ctf[:], pslt)
        cti = gp.tile([NSLOT, NH], I32)
        nc.vector.tensor_copy(cti[:], ctf[:])

        ne_all = gp.tile([NH, NSLOT], F32)
        qgTh = [None] * NH
        for h in range(NH):
            xsel = gbig.tile([128, H], F32, tag="xsel", bufs=1)
            nc.gpsimd.indirect_dma_start(
                out=xsel[0:NSLOT, :], out_offset=None, in_=x_d,
                in_offset=bass.IndirectOffsetOnAxis(ap=cti[:, h:h + 1], axis=0))
            xct = gbig.tile([128, KO, NSLOT], F32, tag="xct", bufs=1)
            for kb in range(KO):
                ptx = psA2k("ptx")[:, :NSLOT]
                nc.tensor.transpose(ptx, xsel[0:NSLOT, ts(kb, 128)],
                                    ident[:NSLOT, :NSLOT])
                nc.vector.tensor_copy(xct[:, kb, :], ptx)
            pqc = psACC("pqc")[:, :NSLOT]
            for kb in range(KO):
                wqf = gw.tile([128, 1, D], F32, tag="wqf")
                nc.sync.dma_start(
                    wqf[:], wq_d.rearrange("(ko p) m -> p ko m", p=128)[:, kb:kb + 1, ts(h, D)])
                nc.tensor.matmul(pqc, wqf[:, 0, :], xct[:, kb, :],
                                 start=(kb == 0), stop=(kb == KO - 1))
            qcf = gw.tile([128, NSLOT], F32, tag="qcf")
            nc.vector.tensor_copy(qcf[:], pqc)
            qgTh[h] = gbig.tile([128, NSLOT], BF16, tag=f"qgT{h}", name=f"qgT{h}")
            nc.vector.tensor_copy(qgTh[h][:], qcf[:])
            sqc = gw.tile([128, NSLOT], F32, tag="sqc")
            nc.vector.tensor_tensor(sqc[:], qcf[:], qcf[:], op=mybir.AluOpType.mult)
            pne = psA2k("pne")[:1, :NSLOT]
            nc.tensor.matmul(pne, ones[:], sqc[:], start=True, stop=True)
            nerow = gw.tile([1, NSLOT], F32, tag="nerow")
            nc.vector.tensor_copy(nerow[:], pne)
            nc.sync.dma_start(ne_all[h:h + 1, :], nerow[:])
        if DEBUG:
            nc.sync.dma_start(dbg["ne"], ne_all[:])

        ne_work = gp.tile([NH, NSLOT], F32)
        nc.vector.tensor_copy(ne_work[:], ne_all[:])
        tops_e = gp.tile([NH, 64], F32)
        for rr in range(8):
            mx = gp.tile([NH, 8], F32, tag="mxe")
            nc.vector.max(out=mx[:], in_=ne_work[:])
            nc.vector.tensor_copy(tops_e[:, ts(rr, 8)], mx[:])
            nc.vector.match_replace(out=ne_work[:], in_to_replace=mx[:],
                                    in_values=ne_work[:], imm_value=-1e30)
        theta = gp.tile([NH, 1], F32)
        nc.vector.tensor_copy(theta[:], tops_e[:, 61:62])

        # sel over the 98 slots; specials (slots 96/97) always selected
        sel = gp.tile([NH, NSLOT], F32)
        nc.vector.tensor_tensor(sel[:], ne_all[:], theta[:].to_broadcast([NH, NSLOT]),
                                op=mybir.AluOpType.is_ge)
        nc.vector.memset(sel[:, NCAND:NSLOT], 1.0)
        # scatter idx per slot: cand_t if selected else OOB (100000)
        sidx_f = gp.tile([NH, NSLOT], F32)
        nc.vector.tensor_scalar(sidx_f[:], sel[:], -1.0, scalar2=None,
                                op0=mybir.AluOpType.add)
        nc.vector.tensor_scalar_mul(sidx_f[:], sidx_f[:], -100000.0)
        nc.vector.tensor_tensor(sidx_f[:], sidx_f[:], cand_t[:], op=mybir.AluOpType.add)
        p_ = psA2k("ptr")[:NSLOT, :NH]
        nc.tensor.transpose(p_, sidx_f[:], ident[:NH, :NH])
        sf1 = gp.tile([NSLOT, NH], F32)
        nc.vector.tensor_copy(sf1[:], p_)
        sidx_i = gp.tile([NSLOT, NH], I32)
        nc.vector.tensor_copy(sidx_i[:], sf1[:])
        if DEBUG:
            nc.sync.dma_start(dbg["sidx"], sidx_i[:])
            nc.sync.dma_start(dbg["na"], nagrid[:])

        for h in range(NH):
            for b in range(A_DONE[0], NB):
                local_block(h, b)
        for h in range(NH):
            global_head(h)

        gw.release()
        gbig.release()
        gp.release()
        ab.release()
        psum.release()
        dram.release()
        res.release()
        const.release()

    nc.finalize()
    return nc


_NC_CACHE = None


def kernel(**inputs):
    global _NC_CACHE
    hs = np.ascontiguousarray(np.asarray(inputs["hidden_states"], dtype=np.float32))
    Wq = np.ascontiguousarray(np.asarray(inputs["Wq"], dtype=np.float32))
    Wk = np.ascontiguousarray(np.asarray(inputs["Wk"], dtype=np.float32))
    Wv = np.ascontiguousarray(np.asarray(inputs["Wv"], dtype=np.float32))
    ident = np.eye(128, dtype=np.float32)

    if _NC_CACHE is None:
        _NC_CACHE = build_program()
    nc = _NC_CACHE
    xts_host = [np.ascontiguousarray(hs[0].T), np.ascontiguousarray(hs[1].T)]

    in_maps = []
    for c in range(8):
        n = c // 4
        h0 = (c % 4) * NH
        cols = slice(h0 * D, (h0 + NH) * D)
        in_maps.append({
            "x": hs[n],
            "xt": xts_host[n],
            "wq": np.ascontiguousarray(Wq[:, cols]),
            "wk": np.ascontiguousarray(Wk[:, cols]),
            "wv": np.ascontiguousarray(Wv[:, cols]),
            "ident": ident,
        })
    res = run_bass_kernel_spmd(nc, in_maps, core_ids=list(range(8)))
    out = np.zeros((2, T, H), np.float32)
    for c in range(8):
        n = c // 4
        h0 = (c % 4) * NH
        for h in range(NH):
            out[n, :, (h0 + h) * D:(h0 + h + 1) * D] = res.results[c][f"out{h}"]
    return out



# revision 13
# speedup vs baseline: 1.0975x; 1.0975x over previous
"""Block-global self-attention Trainium2 kernel (SPMD over 8 NeuronCores).

Sharding: core c -> batch n = c//4, heads h0 = (c%4)*4 .. h0+3.
Each core receives x = hidden[n] [4096,2048] fp32 (gather source),
xt = x^T bf16 [2048,4096] (host-cast), wq/wk/wv bf16 [2048,512]
(head-column stripes, host-cast), wqf fp32 [2048,512] (exact-norm
recompute), returns out [4096,512] fp32.

Per-core pipeline:
  P: bf16 projections from host-cast inputs (no on-chip weight/x casts)
     -> qT/kT [d,t] + V2 (t-major, 64-row-shifted); q-norms -> nagrid
     (direct SBUF writes, no DRAM roundtrip). Local blocks up to b<20
     interleaved once their chunks land (2-chunk lag).
  B: top-96 candidates via packed-value 3-level max8 tournament ->
     indirect-gather x rows -> exact fp32 norms -> 62nd threshold
     (bos/eos forced) -> global attention in Sg^T layout (scores
     transposed so PV uses exp output directly as lhsT; no PE
     transposes) -> indirect row scatter. Remaining local blocks
     interleaved to keep PE busy during the vector-bound topk chain.
"""
import numpy as np
import ml_dtypes

import concourse.bass as bass
import concourse.bacc as bacc
import concourse.mybir as mybir
from concourse.tile import TileContext, add_dep_helper
from concourse.bass_utils import run_bass_kernel_spmd

F32 = mybir.dt.float32
BF16 = mybir.dt.bfloat16
I32 = mybir.dt.int32

T = 4096
H = 2048
D = 128
NH = 4
KO = H // 128
NB = T // 128
CW = 512
NCHUNK = T // CW
NEG = -30.0
NEGRAW = -30.0 * float(np.sqrt(128.0))  # pre-divided by ACT scale
SCALE = float(1.0 / np.sqrt(128.0))
NCAND = 96
NSLOT = NCAND + 2
P_LOCAL_LAG = 8  # local blocks emitted during P: b < 4*c - P_LOCAL_LAG


def ts(i, sz):
    return slice(i * sz, (i + 1) * sz)


def _raw(inst):
    return inst.ins if hasattr(inst, "ins") else inst


def build_program():
    nc = bacc.Bacc("TRN2", target_bir_lowering=False, debug=False,
                   enable_asserts=True)
    x_d = nc.dram_tensor("x", (T, H), F32, kind="ExternalInput").ap()
    xt_d = nc.dram_tensor("xt", (H, T), BF16, kind="ExternalInput").ap()
    wq_d = nc.dram_tensor("wq", (H, NH * D), BF16, kind="ExternalInput").ap()
    wk_d = nc.dram_tensor("wk", (H, NH * D), BF16, kind="ExternalInput").ap()
    wv_d = nc.dram_tensor("wv", (H, NH * D), BF16, kind="ExternalInput").ap()
    wqf_d = nc.dram_tensor("wqf", (H, NH * D), F32, kind="ExternalInput").ap()
    id_d = nc.dram_tensor("ident", (128, 128), F32, kind="ExternalInput").ap()
    out_d = [nc.dram_tensor(f"out{h}", (T, D), F32, kind="ExternalOutput").ap()
             for h in range(NH)]

    with TileContext(nc) as tc:
        const = tc.alloc_tile_pool(name="const", bufs=1)
        res = tc.alloc_tile_pool(name="res", bufs=1)
        dram = tc.alloc_tile_pool(name="dram", bufs=1, space="DRAM")

        ident = const.tile([128, 128], F32)
        nc.sync.dma_start(ident[:], id_d)
        ones_b = const.tile([128, 1], BF16)
        nc.vector.memset(ones_b[:], 1.0)
        ones = const.tile([128, 1], F32)
        nc.vector.memset(ones[:], 1.0)
        iota_g = const.tile([128, NH, 32], F32)
        nc.gpsimd.iota(iota_g[:], pattern=[[0, NH], [1, 32]], base=0,
                       channel_multiplier=32, allow_small_or_imprecise_dtypes=True)

        qT = [res.tile([128, T], BF16, tag=f"qT{h}", name=f"qT{h}") for h in range(NH)]
        kT = [res.tile([128, 64 + T + 64], BF16, tag=f"kT{h}", name=f"kT{h}") for h in range(NH)]
        V2 = res.tile([128, NB + 1, NH, D + 1], BF16, tag="V2")
        nagrid = res.tile([128, NH, 32], F32, tag="nagrid")
        na_dram = dram.tile([NH, T], F32)

        # ---------------- pools ----------------
        psum = tc.alloc_tile_pool(name="psum", bufs=1, space="PSUM")
        ab = tc.alloc_tile_pool(name="ab", bufs=4)

        def psA2k(nm):   # 2KB f32 one-shot psums (proj + misc + global SgT)
            return psum.tile([128, 512], F32, tag="A2k", bufs=2, name=nm)
        def psBLK(nm):   # per-block S + ctx combined (local)
            return psum.tile([128, 512], F32, tag="blk", bufs=2, name=nm)
        def psGA(nm):    # held accumulators, one bank per head (pqc/pgc)
            return psum.tile([128, 512], F32, tag="GAC", bufs=4, name=nm)

        out_write_insts = [[] for _ in range(NH)]

        def local_block(h, b):
            blk = psBLK("blk")
            # S^T halves: [tk(128), tq(128)]; half g covers window pos g*128..,
            # i.e. k tokens [b*128 - 64 + g*128, ...). kT is 64-padded.
            for g in range(2):
                seg = b + g
                nc.tensor.matmul(blk[:, g * 128:(g + 1) * 128],
                                 kT[h][:, seg * 128:seg * 128 + 128],
                                 qT[h][:, ts(b, 128)], start=True, stop=True)
            if b == 0:
                nc.vector.memset(blk[0:64, 0:128], NEGRAW)
            if b == NB - 1:
                nc.vector.memset(blk[64:128, 128:256], NEGRAW)
            PT = ab.tile([128, 256], BF16, tag="PT", name="PT", bufs=2)
            nc.scalar.activation(PT[:], blk[:, 0:256], mybir.ActivationFunctionType.Exp,
                                 scale=SCALE)
            pC = blk[:, 256:385]
            nc.tensor.matmul(pC, PT[:, 0:128], V2[:, b, h, :],
                             start=True, stop=False)
            nc.tensor.matmul(pC, PT[:, 128:256], V2[:, b + 1, h, :],
                             start=False, stop=True)
            rc = ab.tile([128, 1], F32, tag="rc", name="rc", bufs=8)
            nc.vector.reciprocal(rc[:], pC[:, 128:129])
            co = ab.tile([128, 128], F32, tag="co", name="co", bufs=3)
            nc.vector.tensor_scalar_mul(co[:], pC[:, 0:128], rc[:])
            w = nc.sync.dma_start(out_d[h][ts(b, 128), :], co[:])
            out_write_insts[h].append(_raw(w))

        # ---------------- phase P ----------------
        wkv = tc.alloc_tile_pool(name="wkv", bufs=1)
        wqb = wkv.tile([128, KO, NH * D], BF16, tag="wqb")
        wkb = wkv.tile([128, KO, NH * D], BF16, tag="wkb")
        wvb = wkv.tile([128, KO, NH * D], BF16, tag="wvb")
        wb = {"q": wqb, "k": wkb, "v": wvb}

        A_DONE = [0]
        with tc.tile_pool(name="pp", bufs=2) as pp, \
             tc.tile_pool(name="pp1", bufs=1) as pp1:

            # bf16 weights: direct per-kb DMA (kb-major so kb=0 lands first)
            wviews = {nm: wd.rearrange("(ko p) m -> p ko m", p=128)
                      for nm, wd in (("q", wq_d), ("k", wk_d), ("v", wv_d))}
            for kb in range(KO):
                for nm in ("q", "k", "v"):
                    nc.sync.dma_start(wb[nm][:, kb:kb + 1, :],
                                      wviews[nm][:, kb:kb + 1, :])

            for h in range(NH):
                nc.vector.memset(kT[h][:, 0:64], 0.0)
                nc.vector.memset(kT[h][:, 64 + T:], 0.0)
            nc.vector.memset(V2[0:64, 0, :, :], 0.0)
            nc.vector.memset(V2[64:128, NB, :, :], 0.0)
            nc.vector.memset(V2[:, :, :, D:D + 1], 1.0)

            xtr = xt_d.rearrange("(ko p) t -> p ko t", p=128)
            for c in range(NCHUNK):
                xtb = pp1.tile([128, KO, CW], BF16, tag="xtb", bufs=2)
                for kb in range(KO):
                    nc.sync.dma_start(xtb[:, kb:kb + 1, :],
                                      xtr[:, kb:kb + 1, ts(c, CW)])
                na_writes_c = []
                for h in range(NH):
                    for nm, dstT in (("q", qT[h]), ("k", kT[h])):
                        ps = psA2k("psqk")
                        for kb in range(KO):
                            nc.tensor.matmul(ps[:], wb[nm][:, kb, ts(h, D)],
                                             xtb[:, kb, :], start=(kb == 0),
                                             stop=(kb == KO - 1))
                        off = 64 if nm == "k" else 0
                        nc.vector.tensor_copy(dstT[:, off + c * CW:off + (c + 1) * CW], ps[:])
                        if nm == "q":
                            sq = pp.tile([128, CW], BF16, tag="sq", bufs=1)
                            nc.vector.tensor_tensor(sq[:], dstT[:, ts(c, CW)],
                                                    dstT[:, ts(c, CW)],
                                                    op=mybir.AluOpType.mult)
                            pn = psA2k("pn")[:1, :]
                            nc.tensor.matmul(pn, ones_b[:], sq[:],
                                             start=True, stop=True)
                            narow = pp.tile([1, CW], F32, tag="narow", bufs=2)
                            nc.vector.tensor_copy(narow[:], pn)
                            w = nc.sync.dma_start(na_dram[h:h + 1, ts(c, CW)],
                                                  narow[:])
                            na_writes_c.append(_raw(w))
                # bounce chunk's norms back as [16, NH, 32] grid rows
                # (free->partition moves need DRAM addressing)
                r = nc.sync.dma_start(
                    nagrid[16 * c:16 * (c + 1), :, :],
                    na_dram[:, ts(c, CW)].rearrange("h (p j) -> p h j", p=16))
                for w in na_writes_c:
                    add_dep_helper(_raw(r), w, reason="na grid read after writes")
                for s in range(CW // 128):
                    sg = c * (CW // 128) + s
                    pv = psA2k("psv")
                    for kb in range(KO):
                        nc.tensor.matmul(pv[:], xtb[:, kb, ts(s, 128)],
                                         wb["v"][:, kb, :], start=(kb == 0),
                                         stop=(kb == KO - 1))
                    vt = pp.tile([128, NH * D], BF16, tag="vtmp", bufs=1)
                    nc.vector.tensor_copy(vt[:], pv[:])
                    nc.sync.dma_start(V2[64:128, sg, :, 0:D],
                                      vt[0:64, :].rearrange("p (h d) -> p h d", h=NH))
                    nc.sync.dma_start(V2[0:64, sg + 1, :, 0:D],
                                      vt[64:128, :].rearrange("p (h d) -> p h d", h=NH))
                # interleave local blocks with a 2-chunk lag; keep the rest
                # to fill PE during the vector-bound topk chain in phase B
                hi = max(0, min(4 * c - P_LOCAL_LAG, NB))
                for b in range(A_DONE[0], hi):
                    for h in range(NH):
                        local_block(h, b)
                A_DONE[0] = max(A_DONE[0], hi)
        wkv.release()

        # ---------------- phase B part 1: candidates + exact topk ----------------
        gp = tc.alloc_tile_pool(name="gp", bufs=1)

        m0 = gp.tile([128, NH, 32], F32)
        nc.vector.tensor_scalar(m0[:], iota_g[:], 0.0, scalar2=None,
                                op0=mybir.AluOpType.is_equal)
        m1 = gp.tile([128, NH, 32], F32)
        nc.vector.tensor_scalar(m1[:], iota_g[:], 4095.0, scalar2=None,
                                op0=mybir.AluOpType.is_equal)
        nc.vector.tensor_tensor(m0[:], m0[:], m1[:], op=mybir.AluOpType.add)
        nagp = gp.tile([128, NH, 32], F32)
        nc.vector.tensor_tensor(nagp[:], nagrid[:], m0[:], op=mybir.AluOpType.mult)
        nc.vector.tensor_tensor(nagp[:], nagrid[:], nagp[:], op=mybir.AluOpType.subtract)
        nc.vector.tensor_scalar_mul(m0[:], m0[:], 1.0e6)
        nc.vector.tensor_tensor(nagp[:], nagp[:], m0[:], op=mybir.AluOpType.subtract)
        pk = gp.tile([128, NH, 32], F32)
        nc.vector.tensor_scalar_mul(pk[:], nagp[:], 4.0)
        pki = gp.tile([128, NH, 32], I32)
        nc.vector.tensor_copy(pki[:], pk[:])
        nc.vector.tensor_copy(pk[:], pki[:])
        nc.vector.tensor_scalar_mul(pk[:], pk[:], 0.125)
        io16 = gp.tile([128, NH, 32], F32)
        nc.vector.tensor_scalar_mul(io16[:], iota_g[:], 2.0 ** -16)
        nc.vector.tensor_tensor(pk[:], pk[:], io16[:], op=mybir.AluOpType.add)
        pk2 = pk[:].rearrange("p h j -> p (h j)")

        cand1 = gp.tile([128, NH * 16], F32)
        for h in range(NH):
            for rr in range(2):
                mx = gp.tile([128, 8], F32, tag="mx1")
                nc.vector.max(out=mx[:], in_=pk2[:, ts(h, 32)])
                nc.vector.tensor_copy(cand1[:, h * 16 + rr * 8:h * 16 + rr * 8 + 8], mx[:])
                nc.vector.match_replace(out=pk2[:, ts(h, 32)], in_to_replace=mx[:],
                                        in_values=pk2[:, ts(h, 32)], imm_value=-1e30)
        lvl2 = gp.tile([64, 128], F32)
        for h in range(NH):
            for g in range(8):
                nc.sync.dma_start(lvl2[h * 16:(h + 1) * 16, ts(g, 16)],
                                  cand1[16 * g:16 * (g + 1), ts(h, 16)])
        cand2 = gp.tile([64, 24], F32)
        for rr in range(3):
            mx = gp.tile([64, 8], F32, tag="mx2")
            nc.vector.max(out=mx[:], in_=lvl2[:])
            nc.vector.tensor_copy(cand2[:, ts(rr, 8)], mx[:])
            nc.vector.match_replace(out=lvl2[:], in_to_replace=mx[:],
                                    in_values=lvl2[:], imm_value=-1e30)
        c2d = dram.tile([64, 24], F32)
        w2 = nc.sync.dma_start(c2d[:], cand2[:])
        lvl3 = gp.tile([NH, 384], F32)
        r3 = nc.sync.dma_start(lvl3[:],
                               c2d[:].rearrange("(h p) c -> h (p c)", h=NH))
        add_dep_helper(_raw(r3), _raw(w2), reason="lvl3 read after write")
        tops = gp.tile([NH, NCAND], F32)
        for rr in range(12):
            mx = gp.tile([NH, 8], F32, tag="mx3")
            nc.vector.max(out=mx[:], in_=lvl3[:])
            nc.vector.tensor_copy(tops[:, ts(rr, 8)], mx[:])
            nc.vector.match_replace(out=lvl3[:], in_to_replace=mx[:],
                                    in_values=lvl3[:], imm_value=-1e30)

        def decode_t(dst, src, n):
            t1 = gp.tile([NH, n], F32, tag="dec1")
            nc.vector.tensor_scalar_mul(t1[:], src, 8.0)
            t1i = gp.tile([NH, n], I32, tag="dec2")
            nc.vector.tensor_copy(t1i[:], t1[:])
            t1f = gp.tile([NH, n], F32, tag="dec3")
            nc.vector.tensor_copy(t1f[:], t1i[:])
            nc.vector.tensor_tensor(t1[:], t1[:], t1f[:], op=mybir.AluOpType.subtract)
            nc.vector.tensor_scalar_mul(dst, t1[:], 8192.0)

        cand_t = gp.tile([NH, NSLOT], F32)
        decode_t(cand_t[:, 0:NCAND], tops[:], NCAND)
        nc.vector.memset(cand_t[:, NCAND:NCAND + 1], 0.0)
        nc.vector.memset(cand_t[:, NCAND + 1:NSLOT], 4095.0)

        # B pools
        gbig = tc.alloc_tile_pool(name="gbig", bufs=2)
        gw = tc.alloc_tile_pool(name="gw", bufs=2)

        pslt = psA2k("pslt")[:NSLOT, :NH]
        nc.tensor.transpose(pslt, cand_t[:], ident[:NH, :NH])
        ctf = gp.tile([NSLOT, NH], F32)
        nc.vector.tensor_copy(ctf[:], pslt)
        cti = gp.tile([NSLOT, NH], I32)
        nc.vector.tensor_copy(cti[:], ctf[:])

        # PE filler while the topk chain runs on DVE
        def emit_local(nblk):
            hi = min(A_DONE[0] + nblk, NB)
            for b in range(A_DONE[0], hi):
                for h in range(NH):
                    local_block(h, b)
            A_DONE[0] = hi

        emit_local(4)

        # exact fp32 norms for the 98 candidate slots, per head
        ne_all = gp.tile([NH, NSLOT], F32)
        qgTh = [None] * NH
        wqfr = wqf_d.rearrange("(ko p) m -> p ko m", p=128)
        for h in range(NH):
            xsel = gbig.tile([128, H], F32, tag="xsel", bufs=2)
            nc.gpsimd.indirect_dma_start(
                out=xsel[0:NSLOT, :], out_offset=None, in_=x_d,
                in_offset=bass.IndirectOffsetOnAxis(ap=cti[:, h:h + 1], axis=0))
            xct = gbig.tile([128, KO, NSLOT], F32, tag="xct", bufs=2)
            for kb in range(KO):
                ptx = psA2k("ptx")[:, :NSLOT]
                nc.tensor.transpose(ptx, xsel[0:NSLOT, ts(kb, 128)],
                                    ident[:NSLOT, :NSLOT])
                nc.vector.tensor_copy(xct[:, kb, :], ptx)
            pqc = psGA("pqc")[:, :NSLOT]
            for kb in range(KO):
                wqf = gw.tile([128, 1, D], F32, tag="wqf")
                nc.sync.dma_start(wqf[:], wqfr[:, kb:kb + 1, ts(h, D)])
                nc.tensor.matmul(pqc, wqf[:, 0, :], xct[:, kb, :],
                                 start=(kb == 0), stop=(kb == KO - 1))
            qcf = gw.tile([128, NSLOT], F32, tag="qcf")
            nc.vector.tensor_copy(qcf[:], pqc)
            qgTh[h] = gbig.tile([128, NSLOT], BF16, tag=f"qgT{h}", name=f"qgT{h}")
            nc.vector.tensor_copy(qgTh[h][:], qcf[:])
            sqc = gw.tile([128, NSLOT], F32, tag="sqc")
            nc.vector.tensor_tensor(sqc[:], qcf[:], qcf[:], op=mybir.AluOpType.mult)
            pne = psA2k("pne")[:1, :NSLOT]
            nc.tensor.matmul(pne, ones[:], sqc[:], start=True, stop=True)
            nerow = gw.tile([1, NSLOT], F32, tag="nerow")
            nc.vector.tensor_copy(nerow[:], pne)
            nc.sync.dma_start(ne_all[h:h + 1, :], nerow[:])
            emit_local(1)

        # top-62 threshold over exact norms; slots 96/97 forced
        ne_work = gp.tile([NH, NSLOT], F32)
        nc.vector.tensor_copy(ne_work[:], ne_all[:])
        tops_e = gp.tile([NH, 64], F32)
        for rr in range(8):
            mx = gp.tile([NH, 8], F32, tag="mxe")
            nc.vector.max(out=mx[:], in_=ne_work[:])
            nc.vector.tensor_copy(tops_e[:, ts(rr, 8)], mx[:])
            nc.vector.match_replace(out=ne_work[:], in_to_replace=mx[:],
                                    in_values=ne_work[:], imm_value=-1e30)
        theta = gp.tile([NH, 1], F32)
        nc.vector.tensor_copy(theta[:], tops_e[:, 61:62])
        sel = gp.tile([NH, NSLOT], F32)
        nc.vector.tensor_tensor(sel[:], ne_all[:], theta[:].to_broadcast([NH, NSLOT]),
                                op=mybir.AluOpType.is_ge)
        nc.vector.memset(sel[:, NCAND:NSLOT], 1.0)
        sidx_f = gp.tile([NH, NSLOT], F32)
        nc.vector.tensor_scalar(sidx_f[:], sel[:], -1.0, scalar2=None,
                                op0=mybir.AluOpType.add)
        nc.vector.tensor_scalar_mul(sidx_f[:], sidx_f[:], -100000.0)
        nc.vector.tensor_tensor(sidx_f[:], sidx_f[:], cand_t[:], op=mybir.AluOpType.add)
        p_ = psA2k("ptr")[:NSLOT, :NH]
        nc.tensor.transpose(p_, sidx_f[:], ident[:NH, :NH])
        sf1 = gp.tile([NSLOT, NH], F32)
        nc.vector.tensor_copy(sf1[:], p_)
        sidx_i = gp.tile([NSLOT, NH], I32)
        nc.vector.tensor_copy(sidx_i[:], sf1[:])

        # ---------------- phase B part 2: global attention (Sg^T layout) ----
        # Block j covers kT cols [128j, 128j+128) = tokens [128j-64, 128j+64),
        # exactly matching V2[:, j]'s 64-shifted layout. SgT[tok, slot] psums
        # feed exp directly into PgT tiles that serve as PV lhsT — no
        # transposes. Blocks processed in pairs to amortize ACT overhead.
        NPAIR = (NB + 1) // 2  # 16 pairs + final single block
        gco_h = [None] * NH

        def global_head(h):
            # 128-element column strides keep every matmul PSUM target
            # 512B-aligned; one whole bank per head's held accumulator
            pgc = pgc_bank[h][:NSLOT, :D + 1]
            for p in range(NPAIR + 1):
                j0 = 2 * p
                nj = 1 if p == NPAIR else 2
                ps = psA2k("sgt")
                for g in range(nj):
                    nc.tensor.matmul(ps[:, 128 * g:128 * g + NSLOT],
                                     kT[h][:, (j0 + g) * 128:(j0 + g + 1) * 128],
                                     qgTh[h][:], start=True, stop=True)
                pgt = gw.tile([128, 256], BF16, tag="pgt", bufs=3)
                if p == 0:
                    # tokens < 0 are pad: zero their prob rows
                    nc.vector.memset(pgt[0:64, 0:NSLOT], 0.0)
                    nc.scalar.activation(pgt[64:128, 0:NSLOT], ps[64:128, 0:NSLOT],
                                         mybir.ActivationFunctionType.Exp, scale=SCALE)
                    nc.scalar.activation(pgt[:, 128:128 + NSLOT], ps[:, 128:128 + NSLOT],
                                         mybir.ActivationFunctionType.Exp, scale=SCALE)
                elif p == NPAIR:
                    # tokens >= T are pad
                    nc.vector.memset(pgt[64:128, 0:NSLOT], 0.0)
                    nc.scalar.activation(pgt[0:64, 0:NSLOT], ps[0:64, 0:NSLOT],
                                         mybir.ActivationFunctionType.Exp, scale=SCALE)
                else:
                    nc.scalar.activation(pgt[:, 0:NSLOT], ps[:, 0:NSLOT],
                                         mybir.ActivationFunctionType.Exp, scale=SCALE)
                    nc.scalar.activation(pgt[:, 128:128 + NSLOT], ps[:, 128:128 + NSLOT],
                                         mybir.ActivationFunctionType.Exp, scale=SCALE)
                for g in range(nj):
                    nc.tensor.matmul(pgc, pgt[:, 128 * g:128 * g + NSLOT],
                                     V2[:, j0 + g, h, :],
                                     start=(j0 + g == 0), stop=(j0 + g == NB),
                                     skip_group_check=True)
            rcg = gw.tile([NSLOT, 1], F32, tag="rcg")
            nc.vector.reciprocal(rcg[:], pgc[:, D:D + 1])
            # bufs=NH: each head's gco is read only by its end-of-program
            # scatter, so no buffer may be reused before then
            gco = gw.tile([NSLOT, 128], F32, tag="gco", bufs=NH)
            nc.vector.tensor_scalar_mul(gco[:], pgc[:, 0:D], rcg[:])
            gco_h[h] = gco

        pgc_bank = [psGA(f"pgc{h}") for h in range(NH)]
        for h in range(NH):
            global_head(h)
            emit_local(1)
        emit_local(NB)  # any remainder

        for h in range(NH):
            scat = nc.gpsimd.indirect_dma_start(
                out=out_d[h][:],
                out_offset=bass.IndirectOffsetOnAxis(ap=sidx_i[:, h:h + 1], axis=0),
                in_=gco_h[h][:], in_offset=None,
                bounds_check=4095, oob_is_err=False)
            for w in out_write_insts[h]:
                add_dep_helper(_raw(scat), w, reason="scatter after local writes")

        gw.release()
        gbig.release()
        gp.release()
        ab.release()
        psum.release()
        dram.release()
        res.release()
        const.release()

    nc.finalize()
    return nc


_NC_CACHE = None


def make_in_maps(hs, Wq, Wk, Wv):
    ident = np.eye(128, dtype=np.float32)
    bf = ml_dtypes.bfloat16
    xts = [np.ascontiguousarray(hs[0].T.astype(bf)),
           np.ascontiguousarray(hs[1].T.astype(bf))]
    in_maps = []
    for c in range(8):
        n = c // 4
        h0 = (c % 4) * NH
        cols = slice(h0 * D, (h0 + NH) * D)
        in_maps.append({
            "x": hs[n],
            "xt": xts[n],
            "wq": np.ascontiguousarray(Wq[:, cols].astype(bf)),
            "wk": np.ascontiguousarray(Wk[:, cols].astype(bf)),
            "wv": np.ascontiguousarray(Wv[:, cols].astype(bf)),
            "wqf": np.ascontiguousarray(Wq[:, cols]),
            "ident": ident,
        })
    return in_maps


def kernel(**inputs):
    global _NC_CACHE
    hs = np.ascontiguousarray(np.asarray(inputs["hidden_states"], dtype=np.float32))
    Wq = np.ascontiguousarray(np.asarray(inputs["Wq"], dtype=np.float32))
    Wk = np.ascontiguousarray(np.asarray(inputs["Wk"], dtype=np.float32))
    Wv = np.ascontiguousarray(np.asarray(inputs["Wv"], dtype=np.float32))

    if _NC_CACHE is None:
        _NC_CACHE = build_program()
    nc = _NC_CACHE
    in_maps = make_in_maps(hs, Wq, Wk, Wv)
    res = run_bass_kernel_spmd(nc, in_maps, core_ids=list(range(8)))
    out = np.zeros((2, T, H), np.float32)
    for c in range(8):
        n = c // 4
        h0 = (c % 4) * NH
        for h in range(NH):
            out[n, :, (h0 + h) * D:(h0 + h + 1) * D] = res.results[c][f"out{h}"]
    return out


# revision 21
# speedup vs baseline: 1.1184x; 1.0190x over previous
"""Block-global self-attention Trainium2 kernel (SPMD over 8 NeuronCores).

Sharding: core c -> batch n = c//4, heads h0 = (c%4)*4 .. h0+3.
Each core receives x = hidden[n] [4096,2048] fp32 (gather source),
xt = x^T bf16 [2048,4096] (host-cast), wq/wk/wv bf16 [2048,512]
(head-column stripes, host-cast), wqf fp32 [2048,512] (exact-norm
recompute), returns out [4096,512] fp32.

Per-core pipeline:
  P: bf16 projections from host-cast inputs (no on-chip weight/x casts)
     -> qT/kT [d,t] + V2 (t-major, 64-row-shifted); q-norms -> nagrid
     (direct SBUF writes, no DRAM roundtrip). Local blocks up to b<20
     interleaved once their chunks land (2-chunk lag).
  B: top-96 candidates via packed-value 3-level max8 tournament ->
     indirect-gather x rows -> exact fp32 norms -> 62nd threshold
     (bos/eos forced) -> global attention in Sg^T layout (scores
     transposed so PV uses exp output directly as lhsT; no PE
     transposes) -> indirect row scatter. Remaining local blocks
     interleaved to keep PE busy during the vector-bound topk chain.
"""
import numpy as np
import ml_dtypes

import concourse.bass as bass
import concourse.bacc as bacc
import concourse.mybir as mybir
from concourse.tile import TileContext, add_dep_helper
from concourse.bass_utils import run_bass_kernel_spmd

F32 = mybir.dt.float32
BF16 = mybir.dt.bfloat16
I32 = mybir.dt.int32

T = 4096
H = 2048
D = 128
NH = 4
KO = H // 128
NB = T // 128
CW = 512
NCHUNK = T // CW
NEG = -30.0
NEGRAW = -30.0 * float(np.sqrt(128.0))  # pre-divided by ACT scale
SCALE = float(1.0 / np.sqrt(128.0))
NCAND = 96
NSLOT = NCAND + 2
P_LOCAL_LAG = 8  # local blocks emitted during P: b < 4*c - P_LOCAL_LAG


def ts(i, sz):
    return slice(i * sz, (i + 1) * sz)


def _raw(inst):
    return inst.ins if hasattr(inst, "ins") else inst


def build_program():
    nc = bacc.Bacc("TRN2", target_bir_lowering=False, debug=False,
                   enable_asserts=True)
    x_d = nc.dram_tensor("x", (T, H), F32, kind="ExternalInput").ap()
    xt_d = nc.dram_tensor("xt", (H, T), BF16, kind="ExternalInput").ap()
    wq_d = nc.dram_tensor("wq", (H, NH * D), BF16, kind="ExternalInput").ap()
    wk_d = nc.dram_tensor("wk", (H, NH * D), BF16, kind="ExternalInput").ap()
    wv_d = nc.dram_tensor("wv", (H, NH * D), BF16, kind="ExternalInput").ap()
    wqf_d = nc.dram_tensor("wqf", (H, NH * D), F32, kind="ExternalInput").ap()
    id_d = nc.dram_tensor("ident", (128, 128), F32, kind="ExternalInput").ap()
    out_d = [nc.dram_tensor(f"out{h}", (T, D), F32, kind="ExternalOutput").ap()
             for h in range(NH)]

    with TileContext(nc) as tc:
        const = tc.alloc_tile_pool(name="const", bufs=1)
        res = tc.alloc_tile_pool(name="res", bufs=1)
        dram = tc.alloc_tile_pool(name="dram", bufs=1, space="DRAM")

        ident = const.tile([128, 128], F32)
        nc.sync.dma_start(ident[:], id_d)
        ones_b = const.tile([128, 1], BF16)
        nc.vector.memset(ones_b[:], 1.0)
        ones = const.tile([128, 1], F32)
        nc.vector.memset(ones[:], 1.0)
        iota_g = const.tile([128, NH, 32], F32)
        nc.gpsimd.iota(iota_g[:], pattern=[[0, NH], [1, 32]], base=0,
                       channel_multiplier=32, allow_small_or_imprecise_dtypes=True)

        qT = [res.tile([128, T], BF16, tag=f"qT{h}", name=f"qT{h}") for h in range(NH)]
        kT = [res.tile([128, 64 + T + 64], BF16, tag=f"kT{h}", name=f"kT{h}") for h in range(NH)]
        V2 = res.tile([128, NB + 1, NH, D + 1], BF16, tag="V2")
        nagrid = res.tile([128, NH, 32], F32, tag="nagrid")
        na_dram = dram.tile([NH, T], F32)

        # ---------------- pools ----------------
        psum = tc.alloc_tile_pool(name="psum", bufs=1, space="PSUM")
        ab = tc.alloc_tile_pool(name="ab", bufs=4)

        def psA2k(nm):   # 2KB f32 one-shot psums (proj + misc + global SgT)
            return psum.tile([128, 512], F32, tag="A2k", bufs=2, name=nm)
        def psBLK(nm):   # per-block S + ctx combined (local)
            return psum.tile([128, 512], F32, tag="blk", bufs=2, name=nm)
        def psGA(nm):    # held accumulators, one bank per head (pqc/pgc)
            return psum.tile([128, 512], F32, tag="GAC", bufs=4, name=nm)

        out_write_insts = [[] for _ in range(NH)]

        def local_block(h, b):
            blk = psBLK("blk")
            # S^T halves: [tk(128), tq(128)]; half g covers window pos g*128..,
            # i.e. k tokens [b*128 - 64 + g*128, ...). kT is 64-padded.
            for g in range(2):
                seg = b + g
                nc.tensor.matmul(blk[:, g * 128:(g + 1) * 128],
                                 kT[h][:, seg * 128:seg * 128 + 128],
                                 qT[h][:, ts(b, 128)], start=True, stop=True)
            if b == 0:
                nc.vector.memset(blk[0:64, 0:128], NEGRAW)
            if b == NB - 1:
                nc.vector.memset(blk[64:128, 128:256], NEGRAW)
            PT = ab.tile([128, 256], BF16, tag="PT", name="PT", bufs=2)
            nc.scalar.activation(PT[:], blk[:, 0:256], mybir.ActivationFunctionType.Exp,
                                 scale=SCALE)
            pC = blk[:, 256:385]
            nc.tensor.matmul(pC, PT[:, 0:128], V2[:, b, h, :],
                             start=True, stop=False)
            nc.tensor.matmul(pC, PT[:, 128:256], V2[:, b + 1, h, :],
                             start=False, stop=True)
            rc = ab.tile([128, 1], F32, tag="rc", name="rc", bufs=8)
            nc.vector.reciprocal(rc[:], pC[:, 128:129])
            co = ab.tile([128, 128], F32, tag="co", name="co", bufs=3)
            nc.vector.tensor_scalar_mul(co[:], pC[:, 0:128], rc[:])
            w = nc.sync.dma_start(out_d[h][ts(b, 128), :], co[:])
            out_write_insts[h].append(_raw(w))

        # ---------------- phase P ----------------
        wkv = tc.alloc_tile_pool(name="wkv", bufs=1)
        wqb = wkv.tile([128, KO, NH * D], BF16, tag="wqb")
        wkb = wkv.tile([128, KO, NH * D], BF16, tag="wkb")
        wvb = wkv.tile([128, KO, NH * D], BF16, tag="wvb")
        wb = {"q": wqb, "k": wkb, "v": wvb}

        A_DONE = [0]
        with tc.tile_pool(name="pp", bufs=2) as pp, \
             tc.tile_pool(name="pp1", bufs=1) as pp1:

            # bf16 weights via the Pool-engine DMA queue (kb-major, q/k before
            # v) so the 16 sync queues carry only x data and the first
            # projection matmul unblocks after ~one slice each
            wviews = {nm: wd.rearrange("(ko p) m -> p ko m", p=128)
                      for nm, wd in (("q", wq_d), ("k", wk_d), ("v", wv_d))}
            for kb in range(KO):
                for nm in ("q", "k"):
                    nc.gpsimd.dma_start(wb[nm][:, kb:kb + 1, :],
                                        wviews[nm][:, kb:kb + 1, :])
            for kb in range(KO):
                nc.gpsimd.dma_start(wb["v"][:, kb:kb + 1, :],
                                    wviews["v"][:, kb:kb + 1, :])

            for h in range(NH):
                nc.vector.memset(kT[h][:, 0:64], 0.0)
                nc.vector.memset(kT[h][:, 64 + T:], 0.0)
            nc.vector.memset(V2[0:64, 0, :, :], 0.0)
            nc.vector.memset(V2[64:128, NB, :, :], 0.0)
            nc.vector.memset(V2[:, :, :, D:D + 1], 1.0)

            xtr = xt_d.rearrange("(ko p) t -> p ko t", p=128)
            for c in range(NCHUNK):
                xtb = pp1.tile([128, KO, CW], BF16, tag="xtb", bufs=2)
                for kb in range(KO):
                    nc.sync.dma_start(xtb[:, kb:kb + 1, :],
                                      xtr[:, kb:kb + 1, ts(c, CW)])
                na_writes_c = []
                for h in range(NH):
                    for nm, dstT in (("q", qT[h]), ("k", kT[h])):
                        ps = psA2k("psqk")
                        for kb in range(KO):
                            nc.tensor.matmul(ps[:], wb[nm][:, kb, ts(h, D)],
                                             xtb[:, kb, :], start=(kb == 0),
                                             stop=(kb == KO - 1))
                        off = 64 if nm == "k" else 0
                        nc.vector.tensor_copy(dstT[:, off + c * CW:off + (c + 1) * CW], ps[:])
                        if nm == "q":
                            sq = pp.tile([128, CW], BF16, tag="sq", bufs=1)
                            nc.vector.tensor_tensor(sq[:], dstT[:, ts(c, CW)],
                                                    dstT[:, ts(c, CW)],
                                                    op=mybir.AluOpType.mult)
                            pn = psA2k("pn")[:1, :]
                            nc.tensor.matmul(pn, ones_b[:], sq[:],
                                             start=True, stop=True)
                            narow = pp.tile([1, CW], F32, tag="narow", bufs=2)
                            nc.vector.tensor_copy(narow[:], pn)
                            w = nc.sync.dma_start(na_dram[h:h + 1, ts(c, CW)],
                                                  narow[:])
                            na_writes_c.append(_raw(w))
                # bounce chunk's norms back as [16, NH, 32] grid rows
                # (free->partition moves need DRAM addressing)
                r = nc.sync.dma_start(
                    nagrid[16 * c:16 * (c + 1), :, :],
                    na_dram[:, ts(c, CW)].rearrange("h (p j) -> p h j", p=16))
                for w in na_writes_c:
                    add_dep_helper(_raw(r), w, reason="na grid read after writes")
                for s in range(CW // 128):
                    sg = c * (CW // 128) + s
                    pv = psA2k("psv")
                    for kb in range(KO):
                        nc.tensor.matmul(pv[:], xtb[:, kb, ts(s, 128)],
                                         wb["v"][:, kb, :], start=(kb == 0),
                                         stop=(kb == KO - 1))
                    vt = pp.tile([128, NH * D], BF16, tag="vtmp", bufs=1)
                    nc.vector.tensor_copy(vt[:], pv[:])
                    nc.sync.dma_start(V2[64:128, sg, :, 0:D],
                                      vt[0:64, :].rearrange("p (h d) -> p h d", h=NH))
                    nc.sync.dma_start(V2[0:64, sg + 1, :, 0:D],
                                      vt[64:128, :].rearrange("p (h d) -> p h d", h=NH))
                # interleave local blocks with a 2-chunk lag; keep the rest
                # to fill PE during the vector-bound topk chain in phase B
                hi = max(0, min(4 * c - P_LOCAL_LAG, NB))
                for b in range(A_DONE[0], hi):
                    for h in range(NH):
                        local_block(h, b)
                A_DONE[0] = max(A_DONE[0], hi)
        wkv.release()

        # ---------------- phase B part 1: candidates + exact topk ----------------
        gp = tc.alloc_tile_pool(name="gp", bufs=1)

        # all remaining local blocks up front: independent PE/ACT work that
        # overlaps the vector-bound candidate chain, and drains the local
        # exps off the ACT queue before the global phase needs it
        for b in range(A_DONE[0], NB):
            for h in range(NH):
                local_block(h, b)
        A_DONE[0] = NB

        m0 = gp.tile([128, NH, 32], F32)
        nc.vector.tensor_scalar(m0[:], iota_g[:], 0.0, scalar2=None,
                                op0=mybir.AluOpType.is_equal)
        m1 = gp.tile([128, NH, 32], F32)
        nc.vector.tensor_scalar(m1[:], iota_g[:], 4095.0, scalar2=None,
                                op0=mybir.AluOpType.is_equal)
        nc.vector.tensor_tensor(m0[:], m0[:], m1[:], op=mybir.AluOpType.add)
        nagp = gp.tile([128, NH, 32], F32)
        nc.vector.tensor_tensor(nagp[:], nagrid[:], m0[:], op=mybir.AluOpType.mult)
        nc.vector.tensor_tensor(nagp[:], nagrid[:], nagp[:], op=mybir.AluOpType.subtract)
        nc.vector.tensor_scalar_mul(m0[:], m0[:], 1.0e6)
        nc.vector.tensor_tensor(nagp[:], nagp[:], m0[:], op=mybir.AluOpType.subtract)
        pk = gp.tile([128, NH, 32], F32)
        nc.vector.tensor_scalar_mul(pk[:], nagp[:], 4.0)
        pki = gp.tile([128, NH, 32], I32)
        nc.vector.tensor_copy(pki[:], pk[:])
        nc.vector.tensor_copy(pk[:], pki[:])
        nc.vector.tensor_scalar_mul(pk[:], pk[:], 0.125)
        io16 = gp.tile([128, NH, 32], F32)
        nc.vector.tensor_scalar_mul(io16[:], iota_g[:], 2.0 ** -16)
        nc.vector.tensor_tensor(pk[:], pk[:], io16[:], op=mybir.AluOpType.add)
        pk2 = pk[:].rearrange("p h j -> p (h j)")

        cand1 = gp.tile([128, NH * 16], F32)
        for h in range(NH):
            for rr in range(2):
                mx = gp.tile([128, 8], F32, tag="mx1")
                nc.vector.max(out=mx[:], in_=pk2[:, ts(h, 32)])
                nc.vector.tensor_copy(cand1[:, h * 16 + rr * 8:h * 16 + rr * 8 + 8], mx[:])
                nc.vector.match_replace(out=pk2[:, ts(h, 32)], in_to_replace=mx[:],
                                        in_values=pk2[:, ts(h, 32)], imm_value=-1e30)
        lvl2 = gp.tile([64, 128], F32)
        for h in range(NH):
            for g in range(8):
                nc.sync.dma_start(lvl2[h * 16:(h + 1) * 16, ts(g, 16)],
                                  cand1[16 * g:16 * (g + 1), ts(h, 16)])
        cand2 = gp.tile([64, 24], F32)
        for rr in range(3):
            mx = gp.tile([64, 8], F32, tag="mx2")
            nc.vector.max(out=mx[:], in_=lvl2[:])
            nc.vector.tensor_copy(cand2[:, ts(rr, 8)], mx[:])
            nc.vector.match_replace(out=lvl2[:], in_to_replace=mx[:],
                                    in_values=lvl2[:], imm_value=-1e30)
        c2d = dram.tile([64, 24], F32)
        w2 = nc.sync.dma_start(c2d[:], cand2[:])
        lvl3 = gp.tile([NH, 384], F32)
        r3 = nc.sync.dma_start(lvl3[:],
                               c2d[:].rearrange("(h p) c -> h (p c)", h=NH))
        add_dep_helper(_raw(r3), _raw(w2), reason="lvl3 read after write")
        tops = gp.tile([NH, NCAND], F32)
        for rr in range(12):
            mx = gp.tile([NH, 8], F32, tag="mx3")
            nc.vector.max(out=mx[:], in_=lvl3[:])
            nc.vector.tensor_copy(tops[:, ts(rr, 8)], mx[:])
            nc.vector.match_replace(out=lvl3[:], in_to_replace=mx[:],
                                    in_values=lvl3[:], imm_value=-1e30)

        def decode_t(dst, src, n):
            t1 = gp.tile([NH, n], F32, tag="dec1")
            nc.vector.tensor_scalar_mul(t1[:], src, 8.0)
            t1i = gp.tile([NH, n], I32, tag="dec2")
            nc.vector.tensor_copy(t1i[:], t1[:])
            t1f = gp.tile([NH, n], F32, tag="dec3")
            nc.vector.tensor_copy(t1f[:], t1i[:])
            nc.vector.tensor_tensor(t1[:], t1[:], t1f[:], op=mybir.AluOpType.subtract)
            nc.vector.tensor_scalar_mul(dst, t1[:], 8192.0)

        cand_t = gp.tile([NH, NSLOT], F32)
        decode_t(cand_t[:, 0:NCAND], tops[:], NCAND)
        nc.vector.memset(cand_t[:, NCAND:NCAND + 1], 0.0)
        nc.vector.memset(cand_t[:, NCAND + 1:NSLOT], 4095.0)

        # B pools
        gbig = tc.alloc_tile_pool(name="gbig", bufs=2)
        gw = tc.alloc_tile_pool(name="gw", bufs=2)

        pslt = psA2k("pslt")[:NSLOT, :NH]
        nc.tensor.transpose(pslt, cand_t[:], ident[:NH, :NH])
        ctf = gp.tile([NSLOT, NH], F32)
        nc.vector.tensor_copy(ctf[:], pslt)
        cti = gp.tile([NSLOT, NH], I32)
        nc.vector.tensor_copy(cti[:], ctf[:])

        # ---------------- global attention (Sg^T layout) ----------------
        # Block j covers kT cols [128j, 128j+128) = tokens [128j-64, 128j+64),
        # exactly matching V2[:, j]'s 64-shifted layout. SgT[tok, slot] psums
        # feed exp directly into PgT tiles that serve as PV lhsT — no
        # transposes. Blocks processed in pairs to amortize ACT overhead.
        NPAIR = (NB + 1) // 2  # 16 pairs + final single block
        gco_h = [None] * NH

        def global_head(h):
            # 128-element column strides keep every matmul PSUM target
            # 512B-aligned; one whole bank per head's held accumulator
            pgc = pgc_bank[h][:NSLOT, :D + 1]
            for p in range(NPAIR + 1):
                j0 = 2 * p
                nj = 1 if p == NPAIR else 2
                ps = psA2k("sgt")
                for g in range(nj):
                    nc.tensor.matmul(ps[:, 128 * g:128 * g + NSLOT],
                                     kT[h][:, (j0 + g) * 128:(j0 + g + 1) * 128],
                                     qgTh[h][:], start=True, stop=True)
                pgt = gw.tile([128, 256], BF16, tag="pgt", bufs=3)
                if p == 0:
                    # tokens < 0 are pad: zero their prob rows
                    nc.vector.memset(pgt[0:64, 0:NSLOT], 0.0)
                    nc.scalar.activation(pgt[64:128, 0:NSLOT], ps[64:128, 0:NSLOT],
                                         mybir.ActivationFunctionType.Exp, scale=SCALE)
                    nc.scalar.activation(pgt[:, 128:128 + NSLOT], ps[:, 128:128 + NSLOT],
                                         mybir.ActivationFunctionType.Exp, scale=SCALE)
                elif p == NPAIR:
                    # tokens >= T are pad
                    nc.vector.memset(pgt[64:128, 0:NSLOT], 0.0)
                    nc.scalar.activation(pgt[0:64, 0:NSLOT], ps[0:64, 0:NSLOT],
                                         mybir.ActivationFunctionType.Exp, scale=SCALE)
                else:
                    # one act spanning both halves (gap cols hold garbage
                    # exp values nothing reads; saves ACT overhead per pair)
                    nc.scalar.activation(pgt[:, 0:128 + NSLOT], ps[:, 0:128 + NSLOT],
                                         mybir.ActivationFunctionType.Exp, scale=SCALE)
                for g in range(nj):
                    nc.tensor.matmul(pgc, pgt[:, 128 * g:128 * g + NSLOT],
                                     V2[:, j0 + g, h, :],
                                     start=(j0 + g == 0), stop=(j0 + g == NB),
                                     skip_group_check=True)
            rcg = gw.tile([NSLOT, 1], F32, tag="rcg")
            nc.vector.reciprocal(rcg[:], pgc[:, D:D + 1])
            # bufs=NH: each head's gco is read only by its end-of-program
            # scatter, so no buffer may be reused before then
            gco = gw.tile([NSLOT, 128], F32, tag="gco", bufs=NH)
            nc.vector.tensor_scalar_mul(gco[:], pgc[:, 0:D], rcg[:])
            gco_h[h] = gco

        # exact fp32 norms for the 98 candidate slots, per head
        ne_all = gp.tile([NH, NSLOT], F32)
        qgTh = [None] * NH
        wqfr = wqf_d.rearrange("(ko p) m -> p ko m", p=128)
        for h in range(NH):
            xsel = gbig.tile([128, H], F32, tag="xsel", bufs=2)
            nc.gpsimd.indirect_dma_start(
                out=xsel[0:NSLOT, :], out_offset=None, in_=x_d,
                in_offset=bass.IndirectOffsetOnAxis(ap=cti[:, h:h + 1], axis=0))
            xct = gbig.tile([128, KO, NSLOT], F32, tag="xct", bufs=2)
            for kb in range(KO):
                ptx = psA2k("ptx")[:, :NSLOT]
                nc.tensor.transpose(ptx, xsel[0:NSLOT, ts(kb, 128)],
                                    ident[:NSLOT, :NSLOT])
                nc.vector.tensor_copy(xct[:, kb, :], ptx)
            pqc = psGA("pqc")[:, :NSLOT]
            for kb in range(KO):
                wqf = gw.tile([128, 1, D], F32, tag="wqf")
                nc.sync.dma_start(wqf[:], wqfr[:, kb:kb + 1, ts(h, D)])
                nc.tensor.matmul(pqc, wqf[:, 0, :], xct[:, kb, :],
                                 start=(kb == 0), stop=(kb == KO - 1))
            qcf = gw.tile([128, NSLOT], F32, tag="qcf")
            nc.vector.tensor_copy(qcf[:], pqc)
            qgTh[h] = gbig.tile([128, NSLOT], BF16, tag=f"qgT{h}", name=f"qgT{h}")
            nc.vector.tensor_copy(qgTh[h][:], qcf[:])
            sqc = gw.tile([128, NSLOT], F32, tag="sqc")
            nc.vector.tensor_tensor(sqc[:], qcf[:], qcf[:], op=mybir.AluOpType.mult)
            pne = psA2k("pne")[:1, :NSLOT]
            nc.tensor.matmul(pne, ones[:], sqc[:], start=True, stop=True)
            nerow = gw.tile([1, NSLOT], F32, tag="nerow")
            nc.vector.tensor_copy(nerow[:], pne)
            nc.sync.dma_start(ne_all[h:h + 1, :], nerow[:])

        # global compute first (needs only qgT/kT/V2, not the selection);
        # the ne-topk chain below runs on DVE concurrently
        pgc_bank = [psGA(f"pgc{h}") for h in range(NH)]
        for h in range(NH):
            global_head(h)

        # top-62 threshold over exact norms; slots 96/97 forced
        ne_work = gp.tile([NH, NSLOT], F32)
        nc.vector.tensor_copy(ne_work[:], ne_all[:])
        tops_e = gp.tile([NH, 64], F32)
        for rr in range(8):
            mx = gp.tile([NH, 8], F32, tag="mxe")
            nc.vector.max(out=mx[:], in_=ne_work[:])
            nc.vector.tensor_copy(tops_e[:, ts(rr, 8)], mx[:])
            nc.vector.match_replace(out=ne_work[:], in_to_replace=mx[:],
                                    in_values=ne_work[:], imm_value=-1e30)
        theta = gp.tile([NH, 1], F32)
        nc.vector.tensor_copy(theta[:], tops_e[:, 61:62])
        sel = gp.tile([NH, NSLOT], F32)
        nc.vector.tensor_tensor(sel[:], ne_all[:], theta[:].to_broadcast([NH, NSLOT]),
                                op=mybir.AluOpType.is_ge)
        nc.vector.memset(sel[:, NCAND:NSLOT], 1.0)
        sidx_f = gp.tile([NH, NSLOT], F32)
        nc.vector.tensor_scalar(sidx_f[:], sel[:], -1.0, scalar2=None,
                                op0=mybir.AluOpType.add)
        nc.vector.tensor_scalar_mul(sidx_f[:], sidx_f[:], -100000.0)
        nc.vector.tensor_tensor(sidx_f[:], sidx_f[:], cand_t[:], op=mybir.AluOpType.add)
        p_ = psA2k("ptr")[:NSLOT, :NH]
        nc.tensor.transpose(p_, sidx_f[:], ident[:NH, :NH])
        sf1 = gp.tile([NSLOT, NH], F32)
        nc.vector.tensor_copy(sf1[:], p_)
        sidx_i = gp.tile([NSLOT, NH], I32)
        nc.vector.tensor_copy(sidx_i[:], sf1[:])

        for h in range(NH):
            scat = nc.gpsimd.indirect_dma_start(
                out=out_d[h][:],
                out_offset=bass.IndirectOffsetOnAxis(ap=sidx_i[:, h:h + 1], axis=0),
                in_=gco_h[h][:], in_offset=None,
                bounds_check=4095, oob_is_err=False)
            for w in out_write_insts[h]:
                add_dep_helper(_raw(scat), w, reason="scatter after local writes")

        gw.release()
        gbig.release()
        gp.release()
        ab.release()
        psum.release()
        dram.release()
        res.release()
        const.release()

    nc.finalize()
    return nc


_NC_CACHE = None


def make_in_maps(hs, Wq, Wk, Wv):
    ident = np.eye(128, dtype=np.float32)
    bf = ml_dtypes.bfloat16
    xts = [np.ascontiguousarray(hs[0].T.astype(bf)),
           np.ascontiguousarray(hs[1].T.astype(bf))]
    in_maps = []
    for c in range(8):
        n = c // 4
        h0 = (c % 4) * NH
        cols = slice(h0 * D, (h0 + NH) * D)
        in_maps.append({
            "x": hs[n],
            "xt": xts[n],
            "wq": np.ascontiguousarray(Wq[:, cols].astype(bf)),
            "wk": np.ascontiguousarray(Wk[:, cols].astype(bf)),
            "wv": np.ascontiguousarray(Wv[:, cols].astype(bf)),
            "wqf": np.ascontiguousarray(Wq[:, cols]),
            "ident": ident,
        })
    return in_maps


def kernel(**inputs):
    global _NC_CACHE
    hs = np.ascontiguousarray(np.asarray(inputs["hidden_states"], dtype=np.float32))
    Wq = np.ascontiguousarray(np.asarray(inputs["Wq"], dtype=np.float32))
    Wk = np.ascontiguousarray(np.asarray(inputs["Wk"], dtype=np.float32))
    Wv = np.ascontiguousarray(np.asarray(inputs["Wv"], dtype=np.float32))

    if _NC_CACHE is None:
        _NC_CACHE = build_program()
    nc = _NC_CACHE
    in_maps = make_in_maps(hs, Wq, Wk, Wv)
    res = run_bass_kernel_spmd(nc, in_maps, core_ids=list(range(8)))
    out = np.zeros((2, T, H), np.float32)
    for c in range(8):
        n = c // 4
        h0 = (c % 4) * NH
        for h in range(NH):
            out[n, :, (h0 + h) * D:(h0 + h + 1) * D] = res.results[c][f"out{h}"]
    return out


# revision 26
# speedup vs baseline: 1.3240x; 1.1839x over previous
"""Block-global self-attention Trainium2 kernel (SPMD over 8 NeuronCores).

Sharding: core c -> batch n = c//4, heads h0 = (c%4)*4 .. h0+3.
Each core receives x = hidden[n] [4096,2048] fp32 (gather source),
xt = x^T bf16 [2048,4096] (host-cast), wq/wk/wv bf16 [2048,512]
(head-column stripes, host-cast), wqf fp32 [2048,512] (exact-norm
recompute), returns out [4096,512] fp32.

Per-core pipeline:
  P: bf16 projections from host-cast inputs (no on-chip weight/x casts)
     -> qT/kT [d,t] + V2 (t-major, 64-row-shifted); q-norms -> nagrid
     (direct SBUF writes, no DRAM roundtrip). Local blocks up to b<20
     interleaved once their chunks land (2-chunk lag).
  B: top-96 candidates via packed-value 3-level max8 tournament ->
     indirect-gather x rows -> exact fp32 norms -> 62nd threshold
     (bos/eos forced) -> global attention in Sg^T layout (scores
     transposed so PV uses exp output directly as lhsT; no PE
     transposes) -> indirect row scatter. Remaining local blocks
     interleaved to keep PE busy during the vector-bound topk chain.
"""
import numpy as np
import ml_dtypes

import concourse.bass as bass
import concourse.bacc as bacc
import concourse.mybir as mybir
from concourse.tile import TileContext, add_dep_helper
from concourse.bass_utils import run_bass_kernel_spmd

F32 = mybir.dt.float32
BF16 = mybir.dt.bfloat16
I32 = mybir.dt.int32

T = 4096
H = 2048
D = 128
NH = 4
KO = H // 128
NB = T // 128
CW = 512
NCHUNK = T // CW
NEG = -30.0
NEGRAW = -30.0 * float(np.sqrt(128.0))  # pre-divided by ACT scale
SCALE = float(1.0 / np.sqrt(128.0))
NCAND = 96
NSLOT = NCAND + 2
P_LOCAL_LAG = 16  # local blocks emitted during P: b < 4*c - P_LOCAL_LAG


def ts(i, sz):
    return slice(i * sz, (i + 1) * sz)


def _raw(inst):
    return inst.ins if hasattr(inst, "ins") else inst


def build_program():
    nc = bacc.Bacc("TRN2", target_bir_lowering=False, debug=False,
                   enable_asserts=True)
    x_d = nc.dram_tensor("x", (T, H), F32, kind="ExternalInput").ap()
    xt_d = nc.dram_tensor("xt", (H, T), BF16, kind="ExternalInput").ap()
    wq_d = nc.dram_tensor("wq", (H, NH * D), BF16, kind="ExternalInput").ap()
    wk_d = nc.dram_tensor("wk", (H, NH * D), BF16, kind="ExternalInput").ap()
    wv_d = nc.dram_tensor("wv", (H, NH * D), BF16, kind="ExternalInput").ap()
    wqf_d = nc.dram_tensor("wqf", (H, NH * D), F32, kind="ExternalInput").ap()
    id_d = nc.dram_tensor("ident", (128, 128), F32, kind="ExternalInput").ap()
    out_d = [nc.dram_tensor(f"out{h}", (T, D), F32, kind="ExternalOutput").ap()
             for h in range(NH)]

    with TileContext(nc) as tc:
        const = tc.alloc_tile_pool(name="const", bufs=1)
        res = tc.alloc_tile_pool(name="res", bufs=1)
        dram = tc.alloc_tile_pool(name="dram", bufs=1, space="DRAM")

        ident = const.tile([128, 128], F32)
        nc.sync.dma_start(ident[:], id_d)
        ones_b = const.tile([128, 1], BF16)
        nc.vector.memset(ones_b[:], 1.0)
        ones = const.tile([128, 1], F32)
        nc.vector.memset(ones[:], 1.0)
        iota_g = const.tile([128, NH, 32], F32)
        nc.gpsimd.iota(iota_g[:], pattern=[[0, NH], [1, 32]], base=0,
                       channel_multiplier=32, allow_small_or_imprecise_dtypes=True)

        qT = [res.tile([128, T], BF16, tag=f"qT{h}", name=f"qT{h}") for h in range(NH)]
        kT = [res.tile([128, 64 + T + 64], BF16, tag=f"kT{h}", name=f"kT{h}") for h in range(NH)]
        V2 = res.tile([128, NB + 1, NH, D + 1], BF16, tag="V2")
        nagrid = res.tile([128, NH, 32], F32, tag="nagrid")
        na_dram = dram.tile([NH, T], F32)

        # ---------------- pools ----------------
        psum = tc.alloc_tile_pool(name="psum", bufs=1, space="PSUM")
        ab = tc.alloc_tile_pool(name="ab", bufs=4)

        def psA2k(nm):   # 2KB f32 one-shot psums (proj + misc + global SgT)
            return psum.tile([128, 512], F32, tag="A2k", bufs=2, name=nm)
        def psBLK(nm):   # per-block S + ctx combined (local)
            return psum.tile([128, 512], F32, tag="blk", bufs=2, name=nm)
        def psGA(nm):    # held accumulators, one bank per head (pqc/pgc)
            return psum.tile([128, 512], F32, tag="GAC", bufs=4, name=nm)

        out_write_insts = [[] for _ in range(NH)]

        def local_block(h, b):
            blk = psBLK("blk")
            # S^T halves: [tk(128), tq(128)]; half g covers window pos g*128..,
            # i.e. k tokens [b*128 - 64 + g*128, ...). kT is 64-padded.
            for g in range(2):
                seg = b + g
                nc.tensor.matmul(blk[:, g * 128:(g + 1) * 128],
                                 kT[h][:, seg * 128:seg * 128 + 128],
                                 qT[h][:, ts(b, 128)], start=True, stop=True)
            if b == 0:
                nc.vector.memset(blk[0:64, 0:128], NEGRAW)
            if b == NB - 1:
                nc.vector.memset(blk[64:128, 128:256], NEGRAW)
            PT = ab.tile([128, 256], BF16, tag="PT", name="PT", bufs=2)
            nc.scalar.activation(PT[:], blk[:, 0:256], mybir.ActivationFunctionType.Exp,
                                 scale=SCALE)
            pC = blk[:, 256:385]
            nc.tensor.matmul(pC, PT[:, 0:128], V2[:, b, h, :],
                             start=True, stop=False)
            nc.tensor.matmul(pC, PT[:, 128:256], V2[:, b + 1, h, :],
                             start=False, stop=True)
            rc = ab.tile([128, 1], F32, tag="rc", name="rc", bufs=8)
            nc.vector.reciprocal(rc[:], pC[:, 128:129])
            co = ab.tile([128, 128], F32, tag="co", name="co", bufs=3)
            nc.vector.tensor_scalar_mul(co[:], pC[:, 0:128], rc[:])
            w = nc.sync.dma_start(out_d[h][ts(b, 128), :], co[:])
            out_write_insts[h].append(_raw(w))

        # ---------------- phase P ----------------
        wkv = tc.alloc_tile_pool(name="wkv", bufs=1)
        wqb = wkv.tile([128, KO, NH * D], BF16, tag="wqb")
        wkb = wkv.tile([128, KO, NH * D], BF16, tag="wkb")
        wvb = wkv.tile([128, KO, NH * D], BF16, tag="wvb")
        wb = {"q": wqb, "k": wkb, "v": wvb}

        A_DONE = [0]
        with tc.tile_pool(name="pp", bufs=2) as pp, \
             tc.tile_pool(name="pp1", bufs=1) as pp1:

            # bf16 weights via the Pool-engine DMA queue (kb-major, q/k before
            # v) so the 16 sync queues carry only x data and the first
            # projection matmul unblocks after ~one slice each
            wviews = {nm: wd.rearrange("(ko p) m -> p ko m", p=128)
                      for nm, wd in (("q", wq_d), ("k", wk_d), ("v", wv_d))}
            for kb in range(KO):
                for nm in ("q", "k"):
                    nc.gpsimd.dma_start(wb[nm][:, kb:kb + 1, :],
                                        wviews[nm][:, kb:kb + 1, :])
            for kb in range(KO):
                nc.gpsimd.dma_start(wb["v"][:, kb:kb + 1, :],
                                    wviews["v"][:, kb:kb + 1, :])

            for h in range(NH):
                nc.vector.memset(kT[h][:, 0:64], 0.0)
                nc.vector.memset(kT[h][:, 64 + T:], 0.0)
            nc.vector.memset(V2[0:64, 0, :, :], 0.0)
            nc.vector.memset(V2[64:128, NB, :, :], 0.0)
            nc.vector.memset(V2[:, :, :, D:D + 1], 1.0)

            xtr = xt_d.rearrange("(ko p) t -> p ko t", p=128)
            for c in range(NCHUNK):
                xtb = pp1.tile([128, KO, CW], BF16, tag="xtb", bufs=2)
                for kb in range(KO):
                    nc.sync.dma_start(xtb[:, kb:kb + 1, :],
                                      xtr[:, kb:kb + 1, ts(c, CW)])
                na_writes_c = []
                for h in range(NH):
                    for nm, dstT in (("q", qT[h]), ("k", kT[h])):
                        ps = psA2k("psqk")
                        for kb in range(KO):
                            nc.tensor.matmul(ps[:], wb[nm][:, kb, ts(h, D)],
                                             xtb[:, kb, :], start=(kb == 0),
                                             stop=(kb == KO - 1))
                        off = 64 if nm == "k" else 0
                        nc.vector.tensor_copy(dstT[:, off + c * CW:off + (c + 1) * CW], ps[:])
                        if nm == "q":
                            sq = pp.tile([128, CW], BF16, tag="sq", bufs=1)
                            nc.vector.tensor_tensor(sq[:], dstT[:, ts(c, CW)],
                                                    dstT[:, ts(c, CW)],
                                                    op=mybir.AluOpType.mult)
                            pn = psGA("pn")[:1, :]
                            nc.tensor.matmul(pn, ones_b[:], sq[:],
                                             start=True, stop=True)
                            narow = pp.tile([1, CW], F32, tag="narow", bufs=2)
                            nc.vector.tensor_copy(narow[:], pn)
                            w = nc.sync.dma_start(na_dram[h:h + 1, ts(c, CW)],
                                                  narow[:])
                            na_writes_c.append(_raw(w))
                # bounce chunk's norms back as [16, NH, 32] grid rows
                # (free->partition moves need DRAM addressing)
                r = nc.sync.dma_start(
                    nagrid[16 * c:16 * (c + 1), :, :],
                    na_dram[:, ts(c, CW)].rearrange("h (p j) -> p h j", p=16))
                for w in na_writes_c:
                    add_dep_helper(_raw(r), w, reason="na grid read after writes")
                for s in range(CW // 128):
                    sg = c * (CW // 128) + s
                    pv = psA2k("psv")
                    for kb in range(KO):
                        nc.tensor.matmul(pv[:], xtb[:, kb, ts(s, 128)],
                                         wb["v"][:, kb, :], start=(kb == 0),
                                         stop=(kb == KO - 1))
                    vt = pp.tile([128, NH * D], BF16, tag="vtmp", bufs=1)
                    nc.vector.tensor_copy(vt[:], pv[:])
                    nc.sync.dma_start(V2[64:128, sg, :, 0:D],
                                      vt[0:64, :].rearrange("p (h d) -> p h d", h=NH))
                    nc.sync.dma_start(V2[0:64, sg + 1, :, 0:D],
                                      vt[64:128, :].rearrange("p (h d) -> p h d", h=NH))
                # interleave local blocks with a 2-chunk lag; keep the rest
                # to fill PE during the vector-bound topk chain in phase B
                hi = max(0, min(4 * c - P_LOCAL_LAG, NB))
                for b in range(A_DONE[0], hi):
                    for h in range(NH):
                        local_block(h, b)
                A_DONE[0] = max(A_DONE[0], hi)
        wkv.release()

        # ---------------- phase B part 1: candidates + exact topk ----------------
        gp = tc.alloc_tile_pool(name="gp", bufs=1)

        # all remaining local blocks up front: independent PE/ACT work that
        # overlaps the vector-bound candidate chain, and drains the local
        # exps off the ACT queue before the global phase needs it
        for b in range(A_DONE[0], NB):
            for h in range(NH):
                local_block(h, b)
        A_DONE[0] = NB

        m0 = gp.tile([128, NH, 32], F32)
        nc.vector.tensor_scalar(m0[:], iota_g[:], 0.0, scalar2=None,
                                op0=mybir.AluOpType.is_equal)
        m1 = gp.tile([128, NH, 32], F32)
        nc.vector.tensor_scalar(m1[:], iota_g[:], 4095.0, scalar2=None,
                                op0=mybir.AluOpType.is_equal)
        nc.vector.tensor_tensor(m0[:], m0[:], m1[:], op=mybir.AluOpType.add)
        nagp = gp.tile([128, NH, 32], F32)
        nc.vector.tensor_tensor(nagp[:], nagrid[:], m0[:], op=mybir.AluOpType.mult)
        nc.vector.tensor_tensor(nagp[:], nagrid[:], nagp[:], op=mybir.AluOpType.subtract)
        nc.vector.tensor_scalar_mul(m0[:], m0[:], 1.0e6)
        nc.vector.tensor_tensor(nagp[:], nagp[:], m0[:], op=mybir.AluOpType.subtract)
        pk = gp.tile([128, NH, 32], F32)
        nc.vector.tensor_scalar_mul(pk[:], nagp[:], 4.0)
        pki = gp.tile([128, NH, 32], I32)
        nc.vector.tensor_copy(pki[:], pk[:])
        nc.vector.tensor_copy(pk[:], pki[:])
        nc.vector.tensor_scalar_mul(pk[:], pk[:], 0.125)
        io16 = gp.tile([128, NH, 32], F32)
        nc.vector.tensor_scalar_mul(io16[:], iota_g[:], 2.0 ** -16)
        nc.vector.tensor_tensor(pk[:], pk[:], io16[:], op=mybir.AluOpType.add)
        pk2 = pk[:].rearrange("p h j -> p (h j)")

        cand1 = gp.tile([128, NH * 16], F32)
        for h in range(NH):
            for rr in range(2):
                mx = cand1[:, h * 16 + rr * 8:h * 16 + rr * 8 + 8]
                nc.vector.max(out=mx, in_=pk2[:, ts(h, 32)])
                nc.vector.match_replace(out=pk2[:, ts(h, 32)], in_to_replace=mx,
                                        in_values=pk2[:, ts(h, 32)], imm_value=-1e30)
        lvl2 = gp.tile([64, 128], F32)
        for h in range(NH):
            for g in range(8):
                nc.sync.dma_start(lvl2[h * 16:(h + 1) * 16, ts(g, 16)],
                                  cand1[16 * g:16 * (g + 1), ts(h, 16)])
        cand2 = gp.tile([64, 24], F32)
        for rr in range(3):
            mx = cand2[:, ts(rr, 8)]
            nc.vector.max(out=mx, in_=lvl2[:])
            nc.vector.match_replace(out=lvl2[:], in_to_replace=mx,
                                    in_values=lvl2[:], imm_value=-1e30)
        c2d = dram.tile([64, 24], F32)
        w2 = nc.sync.dma_start(c2d[:], cand2[:])
        lvl3 = gp.tile([NH, 384], F32)
        r3 = nc.sync.dma_start(lvl3[:],
                               c2d[:].rearrange("(h p) c -> h (p c)", h=NH))
        add_dep_helper(_raw(r3), _raw(w2), reason="lvl3 read after write")
        tops = gp.tile([NH, NCAND], F32)
        for rr in range(12):
            mx = tops[:, ts(rr, 8)]
            nc.vector.max(out=mx, in_=lvl3[:])
            nc.vector.match_replace(out=lvl3[:], in_to_replace=mx,
                                    in_values=lvl3[:], imm_value=-1e30)

        def decode_t(dst, src, n):
            t1 = gp.tile([NH, n], F32, tag="dec1")
            nc.vector.tensor_scalar_mul(t1[:], src, 8.0)
            t1i = gp.tile([NH, n], I32, tag="dec2")
            nc.vector.tensor_copy(t1i[:], t1[:])
            t1f = gp.tile([NH, n], F32, tag="dec3")
            nc.vector.tensor_copy(t1f[:], t1i[:])
            nc.vector.tensor_tensor(t1[:], t1[:], t1f[:], op=mybir.AluOpType.subtract)
            nc.vector.tensor_scalar_mul(dst, t1[:], 8192.0)

        cand_t = gp.tile([NH, NSLOT], F32)
        decode_t(cand_t[:, 0:NCAND], tops[:], NCAND)
        nc.vector.memset(cand_t[:, NCAND:NCAND + 1], 0.0)
        nc.vector.memset(cand_t[:, NCAND + 1:NSLOT], 4095.0)

        # B pools
        gbig = tc.alloc_tile_pool(name="gbig", bufs=2)
        gw = tc.alloc_tile_pool(name="gw", bufs=2)

        pslt = psA2k("pslt")[:NSLOT, :NH]
        nc.tensor.transpose(pslt, cand_t[:], ident[:NH, :NH])
        ctf = gp.tile([NSLOT, NH], F32)
        nc.vector.tensor_copy(ctf[:], pslt)
        cti = gp.tile([NSLOT, NH], I32)
        nc.vector.tensor_copy(cti[:], ctf[:])

        # exact fp32 norms for the 98 candidate slots, per head; wq fp32
        # preloaded in one burst (overlaps the candidate chain above)
        wqfull = gbig.tile([128, KO, NH * D], F32, tag="wqfull", bufs=1)
        wqfr = wqf_d.rearrange("(ko p) m -> p ko m", p=128)
        for kb in range(KO):
            nc.sync.dma_start(wqfull[:, kb:kb + 1, :], wqfr[:, kb:kb + 1, :])

        ne_all = gp.tile([NH, NSLOT], F32)
        qgTh = [None] * NH
        for h in range(NH):
            xsel = gbig.tile([128, H], F32, tag="xsel", bufs=2)
            nc.gpsimd.indirect_dma_start(
                out=xsel[0:NSLOT, :], out_offset=None, in_=x_d,
                in_offset=bass.IndirectOffsetOnAxis(ap=cti[:, h:h + 1], axis=0))
            xct = gbig.tile([128, KO, NSLOT], F32, tag="xct", bufs=2)
            for kb in range(KO):
                ptx = psA2k("ptx")[:, :NSLOT]
                nc.tensor.transpose(ptx, xsel[0:NSLOT, ts(kb, 128)],
                                    ident[:NSLOT, :NSLOT])
                nc.vector.tensor_copy(xct[:, kb, :], ptx)
            pqc = psGA("pqc")[:, :NSLOT]
            for kb in range(KO):
                nc.tensor.matmul(pqc, wqfull[:, kb, ts(h, D)], xct[:, kb, :],
                                 start=(kb == 0), stop=(kb == KO - 1))
            qcf = gw.tile([128, NSLOT], F32, tag="qcf")
            nc.vector.tensor_copy(qcf[:], pqc)
            qgTh[h] = gbig.tile([128, NSLOT], BF16, tag=f"qgT{h}", name=f"qgT{h}")
            nc.vector.tensor_copy(qgTh[h][:], qcf[:])
            sqc = gw.tile([128, NSLOT], F32, tag="sqc")
            nc.vector.tensor_tensor(sqc[:], qcf[:], qcf[:], op=mybir.AluOpType.mult)
            pne = psA2k("pne")[:1, :NSLOT]
            nc.tensor.matmul(pne, ones[:], sqc[:], start=True, stop=True)
            nerow = gw.tile([1, NSLOT], F32, tag="nerow")
            nc.vector.tensor_copy(nerow[:], pne)
            nc.sync.dma_start(ne_all[h:h + 1, :], nerow[:])

        # top-62 threshold on DVE (emitted before the global phase so the
        # vector queue drains it early; the PE-side transpose comes after)
        ne_work = gp.tile([NH, NSLOT], F32)
        nc.vector.tensor_copy(ne_work[:], ne_all[:])
        tops_e = gp.tile([NH, 64], F32)
        for rr in range(8):
            mx = tops_e[:, ts(rr, 8)]
            nc.vector.max(out=mx, in_=ne_work[:])
            nc.vector.match_replace(out=ne_work[:], in_to_replace=mx,
                                    in_values=ne_work[:], imm_value=-1e30)
        theta = gp.tile([NH, 1], F32)
        nc.vector.tensor_copy(theta[:], tops_e[:, 61:62])
        sel = gp.tile([NH, NSLOT], F32)
        nc.vector.tensor_tensor(sel[:], ne_all[:], theta[:].to_broadcast([NH, NSLOT]),
                                op=mybir.AluOpType.is_ge)
        nc.vector.memset(sel[:, NCAND:NSLOT], 1.0)
        sidx_f = gp.tile([NH, NSLOT], F32)
        nc.vector.tensor_scalar(sidx_f[:], sel[:], -1.0, scalar2=None,
                                op0=mybir.AluOpType.add)
        nc.vector.tensor_scalar_mul(sidx_f[:], sidx_f[:], -100000.0)
        nc.vector.tensor_tensor(sidx_f[:], sidx_f[:], cand_t[:], op=mybir.AluOpType.add)

        # ---------------- global attention (Sg^T layout) ----------------
        # Block j covers kT cols [128j, 128j+128) = tokens [128j-64, 128j+64),
        # exactly matching V2[:, j]'s 64-shifted layout. SgT[tok, slot] psums
        # feed exp directly into PgT tiles that serve as PV lhsT — no
        # transposes. Per pair-step all four heads' SgTs are emitted before
        # any PV so the in-order LDWEIGHTS queue never waits on an exp.
        NPAIR = (NB + 1) // 2  # 16 pairs + final single block
        pgc_bank = [psGA(f"pgc{h}") for h in range(NH)]
        pgc = [pgc_bank[h][:NSLOT, :D + 1] for h in range(NH)]
        for p in range(NPAIR + 1):
            j0 = 2 * p
            nj = 1 if p == NPAIR else 2
            pgts = []
            for h in range(NH):
                ps = psA2k("sgt")
                for g in range(nj):
                    nc.tensor.matmul(ps[:, 128 * g:128 * g + NSLOT],
                                     kT[h][:, (j0 + g) * 128:(j0 + g + 1) * 128],
                                     qgTh[h][:], start=True, stop=True)
                pgt = gw.tile([128, 256], BF16, tag="pgt", bufs=2 * NH)
                if p == 0:
                    # tokens < 0 are pad: zero their prob rows
                    nc.vector.memset(pgt[0:64, 0:NSLOT], 0.0)
                    nc.scalar.activation(pgt[64:128, 0:NSLOT], ps[64:128, 0:NSLOT],
                                         mybir.ActivationFunctionType.Exp, scale=SCALE)
                    nc.scalar.activation(pgt[:, 128:128 + NSLOT], ps[:, 128:128 + NSLOT],
                                         mybir.ActivationFunctionType.Exp, scale=SCALE)
                elif p == NPAIR:
                    # tokens >= T are pad
                    nc.vector.memset(pgt[64:128, 0:NSLOT], 0.0)
                    nc.scalar.activation(pgt[0:64, 0:NSLOT], ps[0:64, 0:NSLOT],
                                         mybir.ActivationFunctionType.Exp, scale=SCALE)
                else:
                    # one act spanning both halves (gap cols hold garbage
                    # exp values nothing reads; saves ACT overhead per pair)
                    nc.scalar.activation(pgt[:, 0:128 + NSLOT], ps[:, 0:128 + NSLOT],
                                         mybir.ActivationFunctionType.Exp, scale=SCALE)
                pgts.append(pgt)
            for h in range(NH):
                for g in range(nj):
                    nc.tensor.matmul(pgc[h], pgts[h][:, 128 * g:128 * g + NSLOT],
                                     V2[:, j0 + g, h, :],
                                     start=(j0 + g == 0), stop=(j0 + g == NB),
                                     skip_group_check=True)
        gco_h = [None] * NH
        for h in range(NH):
            rcg = gw.tile([NSLOT, 1], F32, tag="rcg")
            nc.vector.reciprocal(rcg[:], pgc[h][:, D:D + 1])
            # bufs=NH: each head's gco is read only by its end-of-program
            # scatter, so no buffer may be reused before then
            gco = gw.tile([NSLOT, 128], F32, tag="gco", bufs=NH)
            nc.vector.tensor_scalar_mul(gco[:], pgc[h][:, 0:D], rcg[:])
            gco_h[h] = gco

        p_ = psA2k("ptr")[:NSLOT, :NH]
        nc.tensor.transpose(p_, sidx_f[:], ident[:NH, :NH])
        sf1 = gp.tile([NSLOT, NH], F32)
        nc.vector.tensor_copy(sf1[:], p_)
        sidx_i = gp.tile([NSLOT, NH], I32)
        nc.vector.tensor_copy(sidx_i[:], sf1[:])

        for h in range(NH):
            scat = nc.gpsimd.indirect_dma_start(
                out=out_d[h][:],
                out_offset=bass.IndirectOffsetOnAxis(ap=sidx_i[:, h:h + 1], axis=0),
                in_=gco_h[h][:], in_offset=None,
                bounds_check=4095, oob_is_err=False)
            for w in out_write_insts[h]:
                add_dep_helper(_raw(scat), w, reason="scatter after local writes")

        gw.release()
        gbig.release()
        gp.release()
        ab.release()
        psum.release()
        dram.release()
        res.release()
        const.release()

    nc.finalize()
    return nc


_NC_CACHE = None


def make_in_maps(hs, Wq, Wk, Wv):
    ident = np.eye(128, dtype=np.float32)
    bf = ml_dtypes.bfloat16
    xts = [np.ascontiguousarray(hs[0].T.astype(bf)),
           np.ascontiguousarray(hs[1].T.astype(bf))]
    in_maps = []
    for c in range(8):
        n = c // 4
        h0 = (c % 4) * NH
        cols = slice(h0 * D, (h0 + NH) * D)
        in_maps.append({
            "x": hs[n],
            "xt": xts[n],
            "wq": np.ascontiguousarray(Wq[:, cols].astype(bf)),
            "wk": np.ascontiguousarray(Wk[:, cols].astype(bf)),
            "wv": np.ascontiguousarray(Wv[:, cols].astype(bf)),
            "wqf": np.ascontiguousarray(Wq[:, cols]),
            "ident": ident,
        })
    return in_maps


def kernel(**inputs):
    global _NC_CACHE
    hs = np.ascontiguousarray(np.asarray(inputs["hidden_states"], dtype=np.float32))
    Wq = np.ascontiguousarray(np.asarray(inputs["Wq"], dtype=np.float32))
    Wk = np.ascontiguousarray(np.asarray(inputs["Wk"], dtype=np.float32))
    Wv = np.ascontiguousarray(np.asarray(inputs["Wv"], dtype=np.float32))

    if _NC_CACHE is None:
        _NC_CACHE = build_program()
    nc = _NC_CACHE
    in_maps = make_in_maps(hs, Wq, Wk, Wv)
    res = run_bass_kernel_spmd(nc, in_maps, core_ids=list(range(8)))
    out = np.zeros((2, T, H), np.float32)
    for c in range(8):
        n = c // 4
        h0 = (c % 4) * NH
        for h in range(NH):
            out[n, :, (h0 + h) * D:(h0 + h + 1) * D] = res.results[c][f"out{h}"]
    return out
